# revision 13
# baseline (speedup 1.0000x reference)
"""Trainium2 Bass kernel for nn_Block_64338610094594.

Forgetting-attention transformer block (rmsnorm -> qkv + per-head
data-dependent log-sigmoid forget gate -> decayed causal softmax ->
out-proj + residual -> rmsnorm -> SwiGLU MLP + residual).

Sharding: 8 cores, data-parallel. Core c owns batch b = c//2 and the
query rows of parity p = c%2 (rows p, p+2, ... of that batch). Each
core of a pair computes K/V/logf for its whole batch (redundant), so
attention, out-proj and MLP need no cross-core communication at all.
The fine row-interleave makes the causal-attention work identical on
every core (uniform SPMD program; per-core differences enter only
through data: the xq gather and the causal mask tile).

Matmul strategy: operands live in SBUF as fp32 and are bitcast to
float32r at the matmul call sites (full PE rate at free-dim >= 256,
same bytes as fp32). The forget-gate projection and the cumulative-sum
matmul stay plain fp32 (their error feeds exp() over ~1000 steps).
"""

import numpy as np

import concourse.bass as bass
import concourse.mybir as mybir
import concourse.tile as tile
from concourse import bacc
from concourse import bass_utils

F32 = mybir.dt.float32
F32R = mybir.dt.float32r

B, S, D = 4, 1024, 2048
H, HD = 16, 128
FF = 5461
FFP = 5504          # FF padded to 43*128
NFC = FFP // 128    # 43 ff chunks
EPS = 1e-5
P = 128
NBLK = S // P       # 8 token blocks per batch
SQ = S // 2         # 512 query rows per core
NQB = SQ // P       # 4 query slots
NDK = D // P        # 16 contraction chunks of d_model
NEG = -1.0e30

_NC = None


def _f32r(ap):
    return ap.bitcast(F32R)


def _mm(nc, out, lhsT, rhs, start=True, stop=True, fast=True):
    nc.tensor.matmul(out, lhsT, rhs, start=start, stop=stop)


def _transpose(nc, out_psum, in_sbuf, ident):
    nc.tensor.transpose(out_psum, in_sbuf, ident)


def build_nc():
    nc = bacc.Bacc("TRN2", target_bir_lowering=False, debug=False)

    xb = nc.dram_tensor("xb", [S, D], F32, kind="ExternalInput").ap()
    xq = nc.dram_tensor("xq", [SQ, D], F32, kind="ExternalInput").ap()
    msk = nc.dram_tensor("msk", [P, 256], F32, kind="ExternalInput").ap()
    wq = nc.dram_tensor("wq", [D, D], F32R, kind="ExternalInput").ap()
    wk = nc.dram_tensor("wk", [D, D], F32R, kind="ExternalInput").ap()
    wv = nc.dram_tensor("wv", [D, D], F32R, kind="ExternalInput").ap()
    wo = nc.dram_tensor("wo", [D, D], F32R, kind="ExternalInput").ap()
    wf = nc.dram_tensor("wf", [D, H], F32, kind="ExternalInput").ap()
    bfv = nc.dram_tensor("bfv", [1, H], F32, kind="ExternalInput").ap()
    qn = nc.dram_tensor("qn", [1, HD], F32, kind="ExternalInput").ap()
    kn = nc.dram_tensor("kn", [1, HD], F32, kind="ExternalInput").ap()
    ln1 = nc.dram_tensor("ln1", [1, D], F32, kind="ExternalInput").ap()
    ln2 = nc.dram_tensor("ln2", [1, D], F32, kind="ExternalInput").ap()
    w1 = nc.dram_tensor("w1", [D, FFP], F32R, kind="ExternalInput").ap()
    w2 = nc.dram_tensor("w2", [D, FFP], F32R, kind="ExternalInput").ap()
    w3 = nc.dram_tensor("w3", [FFP, D], F32R, kind="ExternalInput").ap()
    uc = nc.dram_tensor("uc", [S, S], F32, kind="ExternalInput").ap()
    out = nc.dram_tensor("out", [SQ, D], F32, kind="ExternalOutput").ap()

    # [ki, ko, n] views of the d_model-contracting weights
    wq3 = wq.rearrange("(ko ki) n -> ki ko n", ki=P)
    wk3 = wk.rearrange("(ko ki) n -> ki ko n", ki=P)
    wv3 = wv.rearrange("(ko ki) n -> ki ko n", ki=P)
    wo3 = wo.rearrange("(ko ki) n -> ki ko n", ki=P)
    wf3 = wf.rearrange("(ko ki) n -> ki ko n", ki=P)
    w13 = w1.rearrange("(ko ki) n -> ki ko n", ki=P)
    w23 = w2.rearrange("(ko ki) n -> ki ko n", ki=P)

    AT = mybir.ActivationFunctionType
    OP = mybir.AluOpType

    with tile.TileContext(nc) as tc:
        with tc.tile_pool(name="const", bufs=1) as const, \
             tc.tile_pool(name="psmall", bufs=1) as psmall, \
             tc.tile_pool(name="dram", bufs=1, space="DRAM") as dpool:

            ident = const.tile([P, P], F32)
            nc.gpsimd.memset(ident[:], 0.0)
            nc.gpsimd.affine_select(
                out=ident[:], in_=ident[:],
                compare_op=OP.not_equal, fill=1.0,
                base=0, pattern=[[-1, P]], channel_multiplier=1)

            qnb = const.tile([P, HD], F32)
            knb = const.tile([P, HD], F32)
            bfb = const.tile([P, H], F32)
            mtile = const.tile([P, 256], F32)
            nc.sync.dma_start(mtile[:], msk)
            nc.sync.dma_start(qnb[:1, :], qn)
            nc.sync.dma_start(knb[:1, :], kn)
            nc.sync.dma_start(bfb[:1, :], bfv)
            nc.gpsimd.partition_broadcast(qnb[:], qnb[:1, :])
            nc.gpsimd.partition_broadcast(knb[:], knb[:1, :])
            nc.gpsimd.partition_broadcast(bfb[:], bfb[:1, :])

            spn = psmall.tile([P, NBLK, H], F32)   # softplus(-(h@wf+bf))
            csp = psmall.tile([H, S], F32)         # cumsum of spn over tokens

            csp_d = dpool.tile([H, S], F32)         # cumsum staging for bcast
            kT_d = dpool.tile([H, HD, S], F32R)     # k^T per head
            qT_d = dpool.tile([H, HD, SQ], F32R)    # q^T per head
            v_d = dpool.tile([NBLK, P, D], F32R)    # v natural
            x2_d = dpool.tile([SQ, D], F32)        # post-attn residual

            # ---------------- Phase A+B: norms, projections ------------
            def norm_transpose(io, tp, src_ap, dst, nblocks, lnb, tag):
                for blk in range(nblocks):
                    xt = io.tile([P, D], F32, tag=f"xt{tag}")
                    nc.sync.dma_start(xt[:], src_ap[blk * P:(blk + 1) * P, :])
                    ssq = io.tile([P, 1], F32, tag=f"ssq{tag}")
                    hh = io.tile([P, D], F32, tag=f"hh{tag}")
                    # Square output is dead (hh overwritten below); only
                    # the row-sum accumulator matters.
                    nc.scalar.activation(hh[:], xt[:], AT.Square,
                                         accum_out=ssq[:])
                    inv = io.tile([P, 1], F32, tag=f"inv{tag}")
                    nc.vector.tensor_scalar(inv[:], ssq[:], 1.0 / D, EPS,
                                            OP.mult, OP.add)
                    # rsqrt(m) = exp(-0.5 * ln(m)); keeps the ACT engine on
                    # the natural_log_exp table (no Sqrt table swaps).
                    nc.scalar.activation(inv[:], inv[:], AT.Ln)
                    nc.scalar.activation(inv[:], inv[:], AT.Exp, scale=-0.5)
                    nc.vector.tensor_tensor(hh[:], xt[:], lnb[:], OP.mult)
                    nc.vector.tensor_scalar_mul(hh[:], hh[:], inv[:])
                    for kc in range(NDK):
                        pst = tp.tile([P, P], F32, tag=f"pst{tag}")
                        _transpose(nc, pst[:], hh[:, kc * P:(kc + 1) * P], ident)
                        nc.any.tensor_copy(
                            out=dst[:, kc, blk * P:(blk + 1) * P], in_=pst[:])

            with tc.tile_pool(name="lnp", bufs=1) as lnp, \
                 tc.tile_pool(name="hpool", bufs=1) as hpool:
                ln1b = lnp.tile([P, D], F32)
                nc.sync.dma_start(ln1b[:1, :], ln1)
                nc.gpsimd.partition_broadcast(ln1b[:], ln1b[:1, :])

                hT = hpool.tile([P, NDK, S], F32R)
                hTq = hpool.tile([P, NDK, SQ], F32R)

                with tc.tile_pool(name="ioA", bufs=2) as io, \
                     tc.tile_pool(name="tpA", bufs=4, space="PSUM") as tp:
                    norm_transpose(io, tp, xb, hT, NBLK, ln1b, "a")
                    norm_transpose(io, tp, xq, hTq, NQB, ln1b, "b")

                with tc.tile_pool(name="ioB", bufs=3) as io, \
                     tc.tile_pool(name="wB", bufs=2) as wpool, \
                     tc.tile_pool(name="psB", bufs=4, space="PSUM") as ps, \
                     tc.tile_pool(name="tpB", bufs=4, space="PSUM") as tp:

                    # forget gate: logf natural [tok, H], fp32 matmuls
                    wft = lnp.tile([P, NDK, H], F32)
                    nc.sync.dma_start(wft[:], wf3)
                    for blk in range(NBLK):
                        pl = ps.tile([P, 512], F32, tag="psb")
                        for dk in range(NDK):
                            _mm(nc, pl[:, :H], _f32r(hT[:, dk, blk * P:(blk + 1) * P]).bitcast(F32) if False else hT[:, dk, blk * P:(blk + 1) * P].bitcast(F32),
                                wft[:, dk, :], start=(dk == 0),
                                stop=(dk == NDK - 1), fast=False)
                        zt = io.tile([P, H], F32, tag="zt")
                        nc.vector.tensor_tensor(zt[:], pl[:, :H], bfb[:],
                                                OP.add)
                        # softplus(-z) = ln(1 + exp(-z))
                        nc.scalar.activation(zt[:], zt[:], AT.Exp, scale=-1.0)
                        nc.scalar.activation(spn[:, blk, :], zt[:], AT.Ln,
                                             bias=1.0)

                    # cumulative decay: csp[h, i] = sum_{j<=i} spn[j, h]
                    for it in range(2):
                        pc = ps.tile([P, 512], F32, tag="psb")
                        for blk in range(NBLK):
                            ut = io.tile([P, 512], F32, tag="ut")
                            nc.sync.dma_start(
                                ut[:], uc[blk * P:(blk + 1) * P,
                                          it * 512:(it + 1) * 512])
                            _mm(nc, pc[:H, :], spn[:, blk, :], ut[:],
                                start=(blk == 0), stop=(blk == NBLK - 1),
                                fast=False)
                        nc.any.tensor_copy(out=csp[:, it * 512:(it + 1) * 512],
                                           in_=pc[:H, :])
                    nc.sync.dma_start(csp_d[:], csp[:])

                    # q/k/v projections, 512-wide dout tiles (4 heads each)
                    def qk_proj(w3ap, hsrc, nblocks, nwb, dst_dram):
                        for dt in range(4):
                            wt = wpool.tile([P, NDK, 512], F32R, tag="wt")
                            nc.sync.dma_start(
                                wt[:], w3ap[:, :, dt * 512:(dt + 1) * 512])
                            for blk in range(nblocks):
                                pp = ps.tile([P, 512], F32, tag="psb")
                                for dk in range(NDK):
                                    _mm(nc, pp[:],
                                        hsrc[:, dk, blk * P:(blk + 1) * P],
                                        wt[:, dk, :], start=(dk == 0),
                                        stop=(dk == NDK - 1))
                                for hh in range(4):
                                    head = dt * 4 + hh
                                    hs = slice(hh * HD, (hh + 1) * HD)
                                    sqt = io.tile([P, HD], F32, tag="sqt")
                                    ssq = io.tile([P, 1], F32, tag="ssqh")
                                    nc.scalar.activation(sqt[:], pp[:, hs],
                                                         AT.Square,
                                                         accum_out=ssq[:])
                                    inv = io.tile([P, 1], F32, tag="invh")
                                    nc.vector.tensor_scalar(
                                        inv[:], ssq[:], 1.0 / HD, EPS,
                                        OP.mult, OP.add)
                                    nc.scalar.activation(inv[:], inv[:], AT.Ln)
                                    nc.scalar.activation(inv[:], inv[:],
                                                         AT.Exp, scale=-0.5)
                                    qs = io.tile([P, HD], F32, tag="qs")
                                    nc.vector.tensor_scalar_mul(qs[:], pp[:, hs],
                                                                inv[:])
                                    nc.vector.tensor_tensor(qs[:], qs[:],
                                                            nwb[:], OP.mult)
                                    pst = tp.tile([P, P], F32, tag="pstb")
                                    _transpose(nc, pst[:], qs[:], ident)
                                    kst = io.tile([P, P], F32R, tag="kst")
                                    nc.any.tensor_copy(out=kst[:], in_=pst[:])
                                    nc.sync.dma_start(
                                        dst_dram[head, :,
                                                 blk * P:(blk + 1) * P],
                                        kst[:])

                    qk_proj(wq3, hTq, NQB, qnb, qT_d)
                    qk_proj(wk3, hT, NBLK, knb, kT_d)

                    for dt in range(4):
                        wt = wpool.tile([P, NDK, 512], F32R, tag="wt")
                        nc.sync.dma_start(wt[:],
                                          wv3[:, :, dt * 512:(dt + 1) * 512])
                        for blk in range(NBLK):
                            pp = ps.tile([P, 512], F32, tag="psb")
                            for dk in range(NDK):
                                _mm(nc, pp[:], hT[:, dk, blk * P:(blk + 1) * P],
                                    wt[:, dk, :], start=(dk == 0),
                                    stop=(dk == NDK - 1))
                            vs = io.tile([P, 512], F32R, tag="vs")
                            nc.any.tensor_copy(out=vs[:], in_=pp[:])
                            nc.sync.dma_start(
                                v_d[blk, :, dt * 512:(dt + 1) * 512], vs[:])

            # ---------------- Phase C+D: attention + out-proj ----------
            with tc.tile_pool(name="opool", bufs=1) as opool:
                oT = opool.tile([P, H, SQ], F32R)

                with tc.tile_pool(name="ioC", bufs=2) as io, \
                     tc.tile_pool(name="sC", bufs=3) as sc, \
                     tc.tile_pool(name="ptC", bufs=2) as ptp, \
                     tc.tile_pool(name="psC", bufs=3, space="PSUM") as ps, \
                     tc.tile_pool(name="tpC", bufs=2, space="PSUM") as tp, \
                     tc.tile_pool(name="opC", bufs=2, space="PSUM") as op:
                    for head in range(H):
                        crep = io.tile([P, S], F32, tag="crep")
                        nc.sync.dma_start(
                            crep[:], csp_d[head:head + 1, :].broadcast_to((P, S)))
                        kth = io.tile([P, S], F32R, tag="kth")
                        nc.sync.dma_start(kth[:], kT_d[head])
                        qth = io.tile([P, SQ], F32R, tag="qth")
                        nc.sync.dma_start(qth[:], qT_d[head])
                        vh = io.tile([P, NBLK, HD], F32R, tag="vh")
                        nc.sync.dma_start(
                            vh[:], v_d[:, :, head * HD:(head + 1) * HD]
                            .rearrange("b p f -> p b f"))
                        ptb = ptp.tile([P, NBLK, SQ], F32R, tag="ptb")

                        for t in range(NQB):
                            nk = 256 * (t + 1)
                            nparts = (nk + 511) // 512
                            sps = []
                            for prt in range(nparts):
                                w = min(512, nk - prt * 512)
                                pt = ps.tile([P, 512], F32, tag="spsum")
                                _mm(nc, pt[:, :w], qth[:, t * P:(t + 1) * P],
                                    kth[:, prt * 512:prt * 512 + w])
                                sps.append((pt, w))
                            sp = sc.tile([P, S], F32, tag="sprime")
                            for prt, (pt, w) in enumerate(sps):
                                nc.vector.tensor_tensor(
                                    sp[:, prt * 512:prt * 512 + w], pt[:, :w],
                                    crep[:, prt * 512:prt * 512 + w], OP.add)
                            nc.vector.tensor_tensor(
                                sp[:, nk - 256:nk], sp[:, nk - 256:nk],
                                mtile[:], OP.add)
                            mx = io.tile([P, 1], F32, tag="mx")
                            nc.vector.reduce_max(mx[:], sp[:, :nk],
                                                 axis=mybir.AxisListType.X)
                            nc.vector.tensor_scalar_mul(mx[:], mx[:], -1.0)
                            pe = sc.tile([P, S], F32, tag="pexp")
                            rs = io.tile([P, 1], F32, tag="rs")
                            nc.scalar.activation(pe[:, :nk], sp[:, :nk],
                                                 AT.Exp, bias=mx[:],
                                                 accum_out=rs[:])
                            nc.vector.reciprocal(rs[:], rs[:])
                            nc.vector.tensor_scalar_mul(pe[:, :nk],
                                                        pe[:, :nk], rs[:])
                            for kc in range(2 * t + 2):
                                pst = tp.tile([P, P], F32, tag="pstc")
                                _transpose(nc, pst[:],
                                           pe[:, kc * P:(kc + 1) * P], ident)
                                nc.any.tensor_copy(
                                    out=ptb[:, kc, t * P:(t + 1) * P],
                                    in_=pst[:])

                        # Suffix accumulation: key-chunk kc only feeds query
                        # slots t >= kc//2. kc=0 (start=True) clears the whole
                        # bank and covers every column, so unwritten regions
                        # of later chunks are never read.
                        po = op.tile([P, SQ], F32, tag="opsum")
                        for kc in range(NBLK):
                            qlo = (kc // 2) * P
                            _mm(nc, po[:, qlo:], vh[:, kc, :], ptb[:, kc, qlo:],
                                start=(kc == 0), stop=(kc == NBLK - 1))
                        nc.any.tensor_copy(out=oT[:, head, :], in_=po[:])

                with tc.tile_pool(name="ioD", bufs=3) as io, \
                     tc.tile_pool(name="wD", bufs=2) as wpool, \
                     tc.tile_pool(name="psD", bufs=4, space="PSUM") as ps:
                    for dt in range(4):
                        wt = wpool.tile([P, NDK, 512], F32R, tag="wod")
                        nc.sync.dma_start(wt[:],
                                          wo3[:, :, dt * 512:(dt + 1) * 512])
                        for slot in range(NQB):
                            pp = ps.tile([P, 512], F32, tag="psd")
                            for hh in range(H):
                                _mm(nc, pp[:],
                                    oT[:, hh, slot * P:(slot + 1) * P],
                                    wt[:, hh, :], start=(hh == 0),
                                    stop=(hh == H - 1))
                            xqt = io.tile([P, 512], F32, tag="xqt")
                            nc.sync.dma_start(
                                xqt[:], xq[slot * P:(slot + 1) * P,
                                           dt * 512:(dt + 1) * 512])
                            x2t = io.tile([P, 512], F32, tag="x2t")
                            nc.vector.tensor_tensor(x2t[:], pp[:], xqt[:],
                                                    OP.add)
                            nc.sync.dma_start(
                                x2_d[slot * P:(slot + 1) * P,
                                     dt * 512:(dt + 1) * 512], x2t[:])

            # ---------------- Phase E: SwiGLU MLP ----------------------
            with tc.tile_pool(name="mpool", bufs=1) as mpool, \
                 tc.tile_pool(name="ln2p", bufs=1) as ln2p:
                ln2b = ln2p.tile([P, D], F32)
                nc.sync.dma_start(ln2b[:1, :], ln2)
                nc.gpsimd.partition_broadcast(ln2b[:], ln2b[:1, :])

                h2T = mpool.tile([P, NDK, SQ], F32R)
                mT = mpool.tile([P, NFC, SQ], F32R)

                with tc.tile_pool(name="ioE", bufs=2) as io, \
                     tc.tile_pool(name="tpE", bufs=4, space="PSUM") as tp:
                    norm_transpose(io, tp, x2_d, h2T, NQB, ln2b, "e")

                with tc.tile_pool(name="ioF", bufs=3) as io, \
                     tc.tile_pool(name="wE", bufs=2) as wpool, \
                     tc.tile_pool(name="psGU", bufs=2, space="PSUM") as psgu, \
                     tc.tile_pool(name="psY", bufs=1, space="PSUM") as psy:
                    for fc in range(NFC):
                        w1t = wpool.tile([P, NDK, P], F32R, tag="w1t")
                        nc.sync.dma_start(w1t[:], w13[:, :, fc * P:(fc + 1) * P])
                        w2t = wpool.tile([P, NDK, P], F32R, tag="w2t")
                        nc.sync.dma_start(w2t[:], w23[:, :, fc * P:(fc + 1) * P])
                        pg = psgu.tile([P, SQ], F32, tag="pge")
                        for dk in range(NDK):
                            _mm(nc, pg[:], w1t[:, dk, :], h2T[:, dk, :],
                                start=(dk == 0), stop=(dk == NDK - 1))
                        pu = psgu.tile([P, SQ], F32, tag="pue")
                        for dk in range(NDK):
                            _mm(nc, pu[:], w2t[:, dk, :], h2T[:, dk, :],
                                start=(dk == 0), stop=(dk == NDK - 1))
                        gs = io.tile([P, SQ], F32, tag="gse")
                        nc.scalar.activation(gs[:], pg[:], AT.Silu)
                        nc.vector.tensor_tensor(mT[:, fc, :], gs[:], pu[:],
                                                OP.mult)

                    for dt in range(4):
                        yps = [psy.tile([P, 512], F32, tag=f"ype{slot}",
                                        name=f"yps{dt}_{slot}")
                               for slot in range(NQB)]
                        for fc in range(NFC):
                            w3t = wpool.tile([P, 512], F32R, tag="w3t")
                            nc.sync.dma_start(
                                w3t[:], w3[fc * P:(fc + 1) * P,
                                           dt * 512:(dt + 1) * 512])
                            for slot in range(NQB):
                                _mm(nc, yps[slot][:],
                                    mT[:, fc, slot * P:(slot + 1) * P], w3t[:],
                                    start=(fc == 0), stop=(fc == NFC - 1))
                        for slot in range(NQB):
                            x2t = io.tile([P, 512], F32, tag="x2r")
                            nc.sync.dma_start(
                                x2t[:], x2_d[slot * P:(slot + 1) * P,
                                             dt * 512:(dt + 1) * 512])
                            ot = io.tile([P, 512], F32, tag="ote")
                            nc.vector.tensor_tensor(ot[:], yps[slot][:],
                                                    x2t[:], OP.add)
                            nc.sync.dma_start(
                                out[slot * P:(slot + 1) * P,
                                    dt * 512:(dt + 1) * 512], ot[:])

    nc.compile()
    return nc


def _get_nc():
    global _NC
    if _NC is None:
        _NC = build_nc()
    return _NC


def _host_prep(inputs):
    x = np.ascontiguousarray(np.asarray(inputs["x"], dtype=np.float32))
    w1p = np.zeros((D, FFP), np.float32)
    w1p[:, :FF] = inputs["w1"]
    w2p = np.zeros((D, FFP), np.float32)
    w2p[:, :FF] = inputs["w2"]
    w3p = np.zeros((FFP, D), np.float32)
    w3p[:FF, :] = inputs["w3"]
    uconst = np.triu(np.ones((S, S), np.float32))
    masks = []
    for p in range(2):
        r = np.arange(P)[:, None]
        f = np.arange(256)[None, :]
        masks.append(np.where(f <= 2 * r + p, 0.0, NEG).astype(np.float32))
    shared = {
        "wq": np.ascontiguousarray(inputs["wq"], dtype=np.float32),
        "wk": np.ascontiguousarray(inputs["wk"], dtype=np.float32),
        "wv": np.ascontiguousarray(inputs["wv"], dtype=np.float32),
        "wo": np.ascontiguousarray(inputs["wo"], dtype=np.float32),
        "wf": np.ascontiguousarray(inputs["wf"], dtype=np.float32),
        "bfv": np.asarray(inputs["bf"], dtype=np.float32).reshape(1, H),
        "qn": (np.asarray(inputs["qn_w"], dtype=np.float32)
               / np.sqrt(HD).astype(np.float32)).reshape(1, HD),
        "kn": np.asarray(inputs["kn_w"], dtype=np.float32).reshape(1, HD),
        "ln1": np.asarray(inputs["ln1_w"], dtype=np.float32).reshape(1, D),
        "ln2": np.asarray(inputs["ln2_w"], dtype=np.float32).reshape(1, D),
        "w1": w1p, "w2": w2p, "w3": w3p, "uc": uconst,
    }
    in_maps = []
    for c in range(8):
        b, p = c // 2, c % 2
        m = dict(shared)
        m["xb"] = np.ascontiguousarray(x[b])
        m["xq"] = np.ascontiguousarray(x[b][p::2])
        m["msk"] = masks[p]
        in_maps.append(m)
    return in_maps


def run(inputs, trace=False, tmpdir=None):
    nc = _get_nc()
    in_maps = _host_prep(inputs)
    res = bass_utils.run_bass_kernel_spmd(
        nc, in_maps, core_ids=list(range(8)), trace=trace, tmpdir=tmpdir)
    out = np.empty((B, S, D), np.float32)
    for c in range(8):
        b, p = c // 2, c % 2
        out[b, p::2, :] = res.results[c]["out"]
    return out, res


def kernel(**inputs):
    out, _ = run(inputs, trace=False)
    return out


# revision 18
# speedup vs baseline: 1.1407x; 1.1407x over previous
"""Trainium2 Bass kernel for nn_Block_64338610094594.

Forgetting-attention transformer block (rmsnorm -> qkv + per-head
data-dependent log-sigmoid forget gate -> decayed causal softmax ->
out-proj + residual -> rmsnorm -> SwiGLU MLP + residual).

Sharding: 8 cores, data-parallel. Core c owns batch b = c//2 and the
query rows of parity p = c%2 (rows p, p+2, ... of that batch). Each
core of a pair computes K/V/logf for its whole batch (redundant), so
attention, out-proj and MLP need no cross-core communication at all.
The fine row-interleave makes the causal-attention work identical on
every core (uniform SPMD program; per-core differences enter only
through data: the xq gather and the causal mask tile).

Matmul strategy: bf16 operands for all large matmuls (full PE rate,
cheap hidden LDWEIGHTS, half the weight DMA), fp32 PSUM accumulation.
The forget-gate projection and the cumulative-sum matmul stay plain
fp32 (their error integrates over ~1000 decay steps before exp()).
"""

import numpy as np

import concourse.bass as bass
import concourse.mybir as mybir
import concourse.tile as tile
from concourse import bacc
from concourse import bass_utils

F32 = mybir.dt.float32
F32R = mybir.dt.float32r
BF16 = mybir.dt.bfloat16

B, S, D = 4, 1024, 2048
H, HD = 16, 128
FF = 5461
FFP = 5504          # FF padded to 43*128
NFC = FFP // 128    # 43 ff chunks
EPS = 1e-5
P = 128
NBLK = S // P       # 8 token blocks per batch
SQ = S // 2         # 512 query rows per core
NQB = SQ // P       # 4 query slots
NDK = D // P        # 16 contraction chunks of d_model
NEG = -1.0e30

_NC = None


def _f32r(ap):
    return ap.bitcast(F32R)


def _mm(nc, out, lhsT, rhs, start=True, stop=True, fast=True):
    nc.tensor.matmul(out, lhsT, rhs, start=start, stop=stop)


def _transpose(nc, out_psum, in_sbuf, ident):
    nc.tensor.transpose(out_psum, in_sbuf, ident)


def build_nc():
    nc = bacc.Bacc("TRN2", target_bir_lowering=False, debug=False)

    xb = nc.dram_tensor("xb", [S, D], F32, kind="ExternalInput").ap()
    xq = nc.dram_tensor("xq", [SQ, D], F32, kind="ExternalInput").ap()
    msk = nc.dram_tensor("msk", [P, 256], F32, kind="ExternalInput").ap()
    wq = nc.dram_tensor("wq", [D, D], BF16, kind="ExternalInput").ap()
    wk = nc.dram_tensor("wk", [D, D], BF16, kind="ExternalInput").ap()
    wv = nc.dram_tensor("wv", [D, D], BF16, kind="ExternalInput").ap()
    wo = nc.dram_tensor("wo", [D, D], BF16, kind="ExternalInput").ap()
    wf = nc.dram_tensor("wf", [D, H], F32, kind="ExternalInput").ap()
    bfv = nc.dram_tensor("bfv", [1, H], F32, kind="ExternalInput").ap()
    qn = nc.dram_tensor("qn", [1, HD], F32, kind="ExternalInput").ap()
    kn = nc.dram_tensor("kn", [1, HD], F32, kind="ExternalInput").ap()
    ln1 = nc.dram_tensor("ln1", [1, D], F32, kind="ExternalInput").ap()
    ln2 = nc.dram_tensor("ln2", [1, D], F32, kind="ExternalInput").ap()
    w1 = nc.dram_tensor("w1", [D, FFP], BF16, kind="ExternalInput").ap()
    w2 = nc.dram_tensor("w2", [D, FFP], BF16, kind="ExternalInput").ap()
    w3 = nc.dram_tensor("w3", [FFP, D], BF16, kind="ExternalInput").ap()
    uc = nc.dram_tensor("uc", [S, S], F32, kind="ExternalInput").ap()
    out = nc.dram_tensor("out", [SQ, D], F32, kind="ExternalOutput").ap()

    # [ki, ko, n] views of the d_model-contracting weights
    wq3 = wq.rearrange("(ko ki) n -> ki ko n", ki=P)
    wk3 = wk.rearrange("(ko ki) n -> ki ko n", ki=P)
    wv3 = wv.rearrange("(ko ki) n -> ki ko n", ki=P)
    wo3 = wo.rearrange("(ko ki) n -> ki ko n", ki=P)
    wf3 = wf.rearrange("(ko ki) n -> ki ko n", ki=P)
    w13 = w1.rearrange("(ko ki) n -> ki ko n", ki=P)
    w23 = w2.rearrange("(ko ki) n -> ki ko n", ki=P)

    AT = mybir.ActivationFunctionType
    OP = mybir.AluOpType

    with tile.TileContext(nc) as tc:
        with tc.tile_pool(name="const", bufs=1) as const, \
             tc.tile_pool(name="psmall", bufs=1) as psmall, \
             tc.tile_pool(name="dram", bufs=1, space="DRAM") as dpool:

            ident = const.tile([P, P], F32)
            nc.gpsimd.memset(ident[:], 0.0)
            nc.gpsimd.affine_select(
                out=ident[:], in_=ident[:],
                compare_op=OP.not_equal, fill=1.0,
                base=0, pattern=[[-1, P]], channel_multiplier=1)

            qnb = const.tile([P, HD], F32)
            knb = const.tile([P, HD], F32)
            bfb = const.tile([P, H], F32)
            mtile = const.tile([P, 256], F32)
            nc.sync.dma_start(mtile[:], msk)
            nc.sync.dma_start(qnb[:1, :], qn)
            nc.sync.dma_start(knb[:1, :], kn)
            nc.sync.dma_start(bfb[:1, :], bfv)
            nc.gpsimd.partition_broadcast(qnb[:], qnb[:1, :])
            nc.gpsimd.partition_broadcast(knb[:], knb[:1, :])
            nc.gpsimd.partition_broadcast(bfb[:], bfb[:1, :])

            spn = psmall.tile([P, NBLK, H], F32)   # softplus(-(h@wf+bf))
            csp = psmall.tile([H, S], F32)         # cumsum of spn over tokens

            csp_d = dpool.tile([H, S], F32)         # cumsum staging for bcast
            kT_d = dpool.tile([H, HD, S], BF16)     # k^T per head
            qT_d = dpool.tile([H, HD, SQ], BF16)    # q^T per head
            v_d = dpool.tile([NBLK, P, D], BF16)    # v natural
            x2_d = dpool.tile([SQ, D], F32)        # post-attn residual

            # ---------------- Phase A+B: norms, projections ------------
            def norm_transpose(io, tp, src_ap, dst, nblocks, lnb, tag,
                               logf_fn=None):
                for blk in range(nblocks):
                    xt = io.tile([P, D], F32, tag=f"xt{tag}")
                    nc.sync.dma_start(xt[:], src_ap[blk * P:(blk + 1) * P, :])
                    ssq = io.tile([P, 1], F32, tag=f"ssq{tag}")
                    hh = io.tile([P, D], F32, tag=f"hh{tag}")
                    # square+reduce on DVE (ACT Square would thrash the
                    # activation tables against Ln/Exp)
                    nc.vector.tensor_tensor(hh[:], xt[:], xt[:], OP.mult)
                    nc.vector.reduce_sum(ssq[:], hh[:],
                                         axis=mybir.AxisListType.X)
                    inv = io.tile([P, 1], F32, tag=f"inv{tag}")
                    nc.vector.tensor_scalar(inv[:], ssq[:], 1.0 / D, EPS,
                                            OP.mult, OP.add)
                    # rsqrt(m) = exp(-0.5 * ln(m)); keeps the ACT engine on
                    # the natural_log_exp table (no Sqrt table swaps).
                    nc.scalar.activation(inv[:], inv[:], AT.Ln)
                    nc.scalar.activation(inv[:], inv[:], AT.Exp, scale=-0.5)
                    nc.vector.tensor_tensor(hh[:], xt[:], lnb[:], OP.mult)
                    nc.vector.tensor_scalar_mul(hh[:], hh[:], inv[:])
                    hf = None
                    if logf_fn is not None:
                        hf = io.tile([P, NDK, P], F32, tag=f"hf{tag}")
                    for kc in range(NDK):
                        pst = tp.tile([P, P], F32, tag="pstn")
                        _transpose(nc, pst[:], hh[:, kc * P:(kc + 1) * P], ident)
                        nc.any.tensor_copy(
                            out=dst[:, kc, blk * P:(blk + 1) * P], in_=pst[:])
                        if hf is not None:
                            nc.any.tensor_copy(out=hf[:, kc, :], in_=pst[:])
                    if logf_fn is not None:
                        logf_fn(blk, hf)

            with tc.tile_pool(name="lnp", bufs=1) as lnp, \
                 tc.tile_pool(name="hpool", bufs=1) as hpool:
                ln1b = lnp.tile([P, D], F32)
                nc.sync.dma_start(ln1b[:1, :], ln1)
                nc.gpsimd.partition_broadcast(ln1b[:], ln1b[:1, :])

                hT = hpool.tile([P, NDK, S], BF16)
                hTq = hpool.tile([P, NDK, SQ], BF16)

                wft = lnp.tile([P, NDK, H], F32)
                nc.sync.dma_start(wft[:], wf3)

                with tc.tile_pool(name="ioA", bufs=2) as io, \
                     tc.tile_pool(name="tpA", bufs=4, space="PSUM") as tp, \
                     tc.tile_pool(name="psL", bufs=2, space="PSUM") as psl:
                    def logf_fn(blk, hf):
                        # forget gate from the fp32 staging of h^T: the
                        # decay path must stay fp32 (errors integrate over
                        # ~1000 cumsum steps before exp()).
                        pl = psl.tile([P, H], F32, tag="psl")
                        for dk in range(NDK):
                            _mm(nc, pl[:], hf[:, dk, :], wft[:, dk, :],
                                start=(dk == 0), stop=(dk == NDK - 1),
                                fast=False)
                        zt = io.tile([P, H], F32, tag="zt")
                        nc.vector.tensor_tensor(zt[:], pl[:], bfb[:], OP.add)
                        # softplus(-z) = ln(1 + exp(-z))
                        nc.scalar.activation(zt[:], zt[:], AT.Exp, scale=-1.0)
                        nc.scalar.activation(spn[:, blk, :], zt[:], AT.Ln,
                                             bias=1.0)

                    norm_transpose(io, tp, xb, hT, NBLK, ln1b, "a",
                                   logf_fn=logf_fn)
                    norm_transpose(io, tp, xq, hTq, NQB, ln1b, "b")

                with tc.tile_pool(name="ioB", bufs=3) as io, \
                     tc.tile_pool(name="wB", bufs=2) as wpool, \
                     tc.tile_pool(name="psB", bufs=4, space="PSUM") as ps, \
                     tc.tile_pool(name="tpB", bufs=4, space="PSUM") as tp:

                    # cumulative decay: csp[h, i] = sum_{j<=i} spn[j, h]
                    for it in range(2):
                        pc = ps.tile([P, 512], F32, tag="psb")
                        for blk in range(NBLK):
                            ut = io.tile([P, 512], F32, tag="ut")
                            nc.sync.dma_start(
                                ut[:], uc[blk * P:(blk + 1) * P,
                                          it * 512:(it + 1) * 512])
                            _mm(nc, pc[:H, :], spn[:, blk, :], ut[:],
                                start=(blk == 0), stop=(blk == NBLK - 1),
                                fast=False)
                        nc.any.tensor_copy(out=csp[:, it * 512:(it + 1) * 512],
                                           in_=pc[:H, :])
                    nc.sync.dma_start(csp_d[:], csp[:])

                    # q/k/v projections, 512-wide dout tiles (4 heads each)
                    def qk_proj(w3ap, hsrc, nblocks, nwb, dst_dram):
                        for dt in range(4):
                            wt = wpool.tile([P, NDK, 512], BF16, tag="wt")
                            nc.sync.dma_start(
                                wt[:], w3ap[:, :, dt * 512:(dt + 1) * 512])
                            for blk in range(nblocks):
                                pp = ps.tile([P, 512], F32, tag="psb")
                                for dk in range(NDK):
                                    _mm(nc, pp[:],
                                        hsrc[:, dk, blk * P:(blk + 1) * P],
                                        wt[:, dk, :], start=(dk == 0),
                                        stop=(dk == NDK - 1))
                                for hh in range(4):
                                    head = dt * 4 + hh
                                    hs = slice(hh * HD, (hh + 1) * HD)
                                    qs = io.tile([P, HD], F32, tag="qs")
                                    nc.any.tensor_copy(out=qs[:], in_=pp[:, hs])
                                    sqt = io.tile([P, HD], F32, tag="sqt")
                                    ssq = io.tile([P, 1], F32, tag="ssqh")
                                    nc.vector.tensor_tensor(sqt[:], qs[:],
                                                            qs[:], OP.mult)
                                    nc.vector.reduce_sum(
                                        ssq[:], sqt[:],
                                        axis=mybir.AxisListType.X)
                                    inv = io.tile([P, 1], F32, tag="invh")
                                    nc.vector.tensor_scalar(
                                        inv[:], ssq[:], 1.0 / HD, EPS,
                                        OP.mult, OP.add)
                                    nc.scalar.activation(inv[:], inv[:], AT.Ln)
                                    nc.scalar.activation(inv[:], inv[:],
                                                         AT.Exp, scale=-0.5)
                                    nc.vector.tensor_scalar_mul(qs[:], qs[:],
                                                                inv[:])
                                    nc.vector.tensor_tensor(qs[:], qs[:],
                                                            nwb[:], OP.mult)
                                    pst = tp.tile([P, P], F32, tag="pstb")
                                    _transpose(nc, pst[:], qs[:], ident)
                                    kst = io.tile([P, P], BF16, tag="kst")
                                    nc.any.tensor_copy(out=kst[:], in_=pst[:])
                                    nc.sync.dma_start(
                                        dst_dram[head, :,
                                                 blk * P:(blk + 1) * P],
                                        kst[:])

                    qk_proj(wq3, hTq, NQB, qnb, qT_d)
                    qk_proj(wk3, hT, NBLK, knb, kT_d)

                    for dt in range(4):
                        wt = wpool.tile([P, NDK, 512], BF16, tag="wt")
                        nc.sync.dma_start(wt[:],
                                          wv3[:, :, dt * 512:(dt + 1) * 512])
                        for blk in range(NBLK):
                            pp = ps.tile([P, 512], F32, tag="psb")
                            for dk in range(NDK):
                                _mm(nc, pp[:], hT[:, dk, blk * P:(blk + 1) * P],
                                    wt[:, dk, :], start=(dk == 0),
                                    stop=(dk == NDK - 1))
                            vs = io.tile([P, 512], BF16, tag="vs")
                            nc.any.tensor_copy(out=vs[:], in_=pp[:])
                            nc.sync.dma_start(
                                v_d[blk, :, dt * 512:(dt + 1) * 512], vs[:])

            # ---------------- Phase C+D: attention + out-proj ----------
            with tc.tile_pool(name="opool", bufs=1) as opool:
                oT = opool.tile([P, H, SQ], BF16)

                with tc.tile_pool(name="ioC", bufs=2) as io, \
                     tc.tile_pool(name="sC", bufs=3) as sc, \
                     tc.tile_pool(name="ptC", bufs=2) as ptp, \
                     tc.tile_pool(name="psC", bufs=3, space="PSUM") as ps, \
                     tc.tile_pool(name="tpC", bufs=2, space="PSUM") as tp, \
                     tc.tile_pool(name="opC", bufs=2, space="PSUM") as op:
                    for head in range(H):
                        crep = io.tile([P, S], F32, tag="crep")
                        nc.sync.dma_start(
                            crep[:], csp_d[head:head + 1, :].broadcast_to((P, S)))
                        kth = io.tile([P, S], BF16, tag="kth")
                        nc.sync.dma_start(kth[:], kT_d[head])
                        qth = io.tile([P, SQ], BF16, tag="qth")
                        nc.sync.dma_start(qth[:], qT_d[head])
                        vh = io.tile([P, NBLK, HD], BF16, tag="vh")
                        nc.sync.dma_start(
                            vh[:], v_d[:, :, head * HD:(head + 1) * HD]
                            .rearrange("b p f -> p b f"))
                        ptb = ptp.tile([P, NBLK, SQ], BF16, tag="ptb")

                        for t in range(NQB):
                            nk = 256 * (t + 1)
                            nparts = (nk + 511) // 512
                            sps = []
                            for prt in range(nparts):
                                w = min(512, nk - prt * 512)
                                pt = ps.tile([P, 512], F32, tag="spsum")
                                _mm(nc, pt[:, :w], qth[:, t * P:(t + 1) * P],
                                    kth[:, prt * 512:prt * 512 + w])
                                sps.append((pt, w))
                            sp = sc.tile([P, S], F32, tag="sprime")
                            for prt, (pt, w) in enumerate(sps):
                                nc.vector.tensor_tensor(
                                    sp[:, prt * 512:prt * 512 + w], pt[:, :w],
                                    crep[:, prt * 512:prt * 512 + w], OP.add)
                            nc.vector.tensor_tensor(
                                sp[:, nk - 256:nk], sp[:, nk - 256:nk],
                                mtile[:], OP.add)
                            mx = io.tile([P, 1], F32, tag="mx")
                            nc.vector.reduce_max(mx[:], sp[:, :nk],
                                                 axis=mybir.AxisListType.X)
                            nc.vector.tensor_scalar_mul(mx[:], mx[:], -1.0)
                            pe = sc.tile([P, S], F32, tag="pexp")
                            rs = io.tile([P, 1], F32, tag="rs")
                            nc.scalar.activation(pe[:, :nk], sp[:, :nk],
                                                 AT.Exp, bias=mx[:],
                                                 accum_out=rs[:])
                            nc.vector.reciprocal(rs[:], rs[:])
                            nc.vector.tensor_scalar_mul(pe[:, :nk],
                                                        pe[:, :nk], rs[:])
                            for kc in range(2 * t + 2):
                                pst = tp.tile([P, P], F32, tag="pstc")
                                _transpose(nc, pst[:],
                                           pe[:, kc * P:(kc + 1) * P], ident)
                                nc.any.tensor_copy(
                                    out=ptb[:, kc, t * P:(t + 1) * P],
                                    in_=pst[:])

                        # Suffix accumulation: key-chunk kc only feeds query
                        # slots t >= kc//2. kc=0 (start=True) clears the whole
                        # bank and covers every column, so unwritten regions
                        # of later chunks are never read.
                        po = op.tile([P, SQ], F32, tag="opsum")
                        for kc in range(NBLK):
                            qlo = (kc // 2) * P
                            _mm(nc, po[:, qlo:], vh[:, kc, :], ptb[:, kc, qlo:],
                                start=(kc == 0), stop=(kc == NBLK - 1))
                        nc.any.tensor_copy(out=oT[:, head, :], in_=po[:])

                with tc.tile_pool(name="ioD", bufs=3) as io, \
                     tc.tile_pool(name="wD", bufs=2) as wpool, \
                     tc.tile_pool(name="psD", bufs=4, space="PSUM") as ps:
                    for dt in range(4):
                        wt = wpool.tile([P, NDK, 512], BF16, tag="wod")
                        nc.sync.dma_start(wt[:],
                                          wo3[:, :, dt * 512:(dt + 1) * 512])
                        for slot in range(NQB):
                            pp = ps.tile([P, 512], F32, tag="psd")
                            for hh in range(H):
                                _mm(nc, pp[:],
                                    oT[:, hh, slot * P:(slot + 1) * P],
                                    wt[:, hh, :], start=(hh == 0),
                                    stop=(hh == H - 1))
                            xqt = io.tile([P, 512], F32, tag="xqt")
                            nc.sync.dma_start(
                                xqt[:], xq[slot * P:(slot + 1) * P,
                                           dt * 512:(dt + 1) * 512])
                            x2t = io.tile([P, 512], F32, tag="x2t")
                            nc.vector.tensor_tensor(x2t[:], pp[:], xqt[:],
                                                    OP.add)
                            nc.sync.dma_start(
                                x2_d[slot * P:(slot + 1) * P,
                                     dt * 512:(dt + 1) * 512], x2t[:])

            # ---------------- Phase E: SwiGLU MLP ----------------------
            with tc.tile_pool(name="mpool", bufs=1) as mpool, \
                 tc.tile_pool(name="ln2p", bufs=1) as ln2p:
                ln2b = ln2p.tile([P, D], F32)
                nc.sync.dma_start(ln2b[:1, :], ln2)
                nc.gpsimd.partition_broadcast(ln2b[:], ln2b[:1, :])

                h2T = mpool.tile([P, NDK, SQ], BF16)
                mT = mpool.tile([P, NFC, SQ], BF16)

                with tc.tile_pool(name="ioE", bufs=2) as io, \
                     tc.tile_pool(name="tpE", bufs=4, space="PSUM") as tp:
                    norm_transpose(io, tp, x2_d, h2T, NQB, ln2b, "e")

                with tc.tile_pool(name="ioF", bufs=3) as io, \
                     tc.tile_pool(name="wE", bufs=2) as wpool, \
                     tc.tile_pool(name="psGU", bufs=2, space="PSUM") as psgu, \
                     tc.tile_pool(name="psY", bufs=1, space="PSUM") as psy:
                    for fc in range(NFC):
                        w1t = wpool.tile([P, NDK, P], BF16, tag="w1t")
                        nc.sync.dma_start(w1t[:], w13[:, :, fc * P:(fc + 1) * P])
                        w2t = wpool.tile([P, NDK, P], BF16, tag="w2t")
                        nc.sync.dma_start(w2t[:], w23[:, :, fc * P:(fc + 1) * P])
                        pg = psgu.tile([P, SQ], F32, tag="pge")
                        for dk in range(NDK):
                            _mm(nc, pg[:], w1t[:, dk, :], h2T[:, dk, :],
                                start=(dk == 0), stop=(dk == NDK - 1))
                        pu = psgu.tile([P, SQ], F32, tag="pue")
                        for dk in range(NDK):
                            _mm(nc, pu[:], w2t[:, dk, :], h2T[:, dk, :],
                                start=(dk == 0), stop=(dk == NDK - 1))
                        gs = io.tile([P, SQ], F32, tag="gse")
                        nc.scalar.activation(gs[:], pg[:], AT.Silu)
                        nc.vector.tensor_tensor(mT[:, fc, :], gs[:], pu[:],
                                                OP.mult)

                    for dt in range(4):
                        yps = [psy.tile([P, 512], F32, tag=f"ype{slot}",
                                        name=f"yps{dt}_{slot}")
                               for slot in range(NQB)]
                        for fc in range(NFC):
                            w3t = wpool.tile([P, 512], BF16, tag="w3t")
                            nc.sync.dma_start(
                                w3t[:], w3[fc * P:(fc + 1) * P,
                                           dt * 512:(dt + 1) * 512])
                            for slot in range(NQB):
                                _mm(nc, yps[slot][:],
                                    mT[:, fc, slot * P:(slot + 1) * P], w3t[:],
                                    start=(fc == 0), stop=(fc == NFC - 1))
                        for slot in range(NQB):
                            x2t = io.tile([P, 512], F32, tag="x2r")
                            nc.sync.dma_start(
                                x2t[:], x2_d[slot * P:(slot + 1) * P,
                                             dt * 512:(dt + 1) * 512])
                            ot = io.tile([P, 512], F32, tag="ote")
                            nc.vector.tensor_tensor(ot[:], yps[slot][:],
                                                    x2t[:], OP.add)
                            nc.sync.dma_start(
                                out[slot * P:(slot + 1) * P,
                                    dt * 512:(dt + 1) * 512], ot[:])

    nc.compile()
    return nc


def _get_nc():
    global _NC
    if _NC is None:
        _NC = build_nc()
    return _NC


def _host_prep(inputs):
    import ml_dtypes
    bf = ml_dtypes.bfloat16
    x = np.ascontiguousarray(np.asarray(inputs["x"], dtype=np.float32))
    w1p = np.zeros((D, FFP), bf)
    w1p[:, :FF] = np.asarray(inputs["w1"]).astype(bf)
    w2p = np.zeros((D, FFP), bf)
    w2p[:, :FF] = np.asarray(inputs["w2"]).astype(bf)
    w3p = np.zeros((FFP, D), bf)
    w3p[:FF, :] = np.asarray(inputs["w3"]).astype(bf)
    uconst = np.triu(np.ones((S, S), np.float32))
    masks = []
    for p in range(2):
        r = np.arange(P)[:, None]
        f = np.arange(256)[None, :]
        masks.append(np.where(f <= 2 * r + p, 0.0, NEG).astype(np.float32))
    shared = {
        "wq": np.asarray(inputs["wq"]).astype(bf),
        "wk": np.asarray(inputs["wk"]).astype(bf),
        "wv": np.asarray(inputs["wv"]).astype(bf),
        "wo": np.asarray(inputs["wo"]).astype(bf),
        "wf": np.ascontiguousarray(inputs["wf"], dtype=np.float32),
        "bfv": np.asarray(inputs["bf"], dtype=np.float32).reshape(1, H),
        "qn": (np.asarray(inputs["qn_w"], dtype=np.float32)
               / np.sqrt(HD).astype(np.float32)).reshape(1, HD),
        "kn": np.asarray(inputs["kn_w"], dtype=np.float32).reshape(1, HD),
        "ln1": np.asarray(inputs["ln1_w"], dtype=np.float32).reshape(1, D),
        "ln2": np.asarray(inputs["ln2_w"], dtype=np.float32).reshape(1, D),
        "w1": w1p, "w2": w2p, "w3": w3p, "uc": uconst,
    }
    in_maps = []
    for c in range(8):
        b, p = c // 2, c % 2
        m = dict(shared)
        m["xb"] = np.ascontiguousarray(x[b])
        m["xq"] = np.ascontiguousarray(x[b][p::2])
        m["msk"] = masks[p]
        in_maps.append(m)
    return in_maps


def run(inputs, trace=False, tmpdir=None):
    nc = _get_nc()
    in_maps = _host_prep(inputs)
    res = bass_utils.run_bass_kernel_spmd(
        nc, in_maps, core_ids=list(range(8)), trace=trace, tmpdir=tmpdir)
    out = np.empty((B, S, D), np.float32)
    for c in range(8):
        b, p = c // 2, c % 2
        out[b, p::2, :] = res.results[c]["out"]
    return out, res


def kernel(**inputs):
    out, _ = run(inputs, trace=False)
    return out


# revision 19
# speedup vs baseline: 1.4967x; 1.3121x over previous
"""Trainium2 Bass kernel for nn_Block_64338610094594.

Forgetting-attention transformer block (rmsnorm -> qkv + per-head
data-dependent log-sigmoid forget gate -> decayed causal softmax ->
out-proj + residual -> rmsnorm -> SwiGLU MLP + residual).

Sharding: 8 cores, data-parallel. Core c owns batch b = c//2 and the
query rows of parity p = c%2 (rows p, p+2, ... of that batch). Each
core of a pair computes K/V/logf for its whole batch (redundant), so
attention, out-proj and MLP need no cross-core communication at all.
The fine row-interleave makes the causal-attention work identical on
every core (uniform SPMD program; per-core differences enter only
through data: the xq gather and the causal mask tile).

Matmul strategy: bf16 operands for all large matmuls (full PE rate,
cheap hidden LDWEIGHTS, half the weight DMA), fp32 PSUM accumulation.
The forget-gate projection and the cumulative-sum matmul stay plain
fp32 (their error integrates over ~1000 decay steps before exp()).
"""

import numpy as np

import concourse.bass as bass
import concourse.mybir as mybir
import concourse.tile as tile
from concourse import bacc
from concourse import bass_utils

F32 = mybir.dt.float32
F32R = mybir.dt.float32r
BF16 = mybir.dt.bfloat16

B, S, D = 4, 1024, 2048
H, HD = 16, 128
FF = 5461
FFP = 5504          # FF padded to 43*128
NFC = FFP // 128    # 43 ff chunks
EPS = 1e-5
P = 128
NBLK = S // P       # 8 token blocks per batch
SQ = S // 2         # 512 query rows per core
NQB = SQ // P       # 4 query slots
NDK = D // P        # 16 contraction chunks of d_model
NEG = -1.0e30

_NC = None


def _f32r(ap):
    return ap.bitcast(F32R)


def _mm(nc, out, lhsT, rhs, start=True, stop=True, fast=True):
    nc.tensor.matmul(out, lhsT, rhs, start=start, stop=stop)


def _transpose(nc, out_psum, in_sbuf, ident):
    nc.tensor.transpose(out_psum, in_sbuf, ident)


def build_nc():
    nc = bacc.Bacc("TRN2", target_bir_lowering=False, debug=False)

    xb = nc.dram_tensor("xb", [S, D], F32, kind="ExternalInput").ap()
    xq = nc.dram_tensor("xq", [SQ, D], F32, kind="ExternalInput").ap()
    msk = nc.dram_tensor("msk", [P, 256], F32, kind="ExternalInput").ap()
    wq = nc.dram_tensor("wq", [D, D], BF16, kind="ExternalInput").ap()
    wk = nc.dram_tensor("wk", [D, D], BF16, kind="ExternalInput").ap()
    wv = nc.dram_tensor("wv", [D, D], BF16, kind="ExternalInput").ap()
    wo = nc.dram_tensor("wo", [D, D], BF16, kind="ExternalInput").ap()
    wf = nc.dram_tensor("wf", [D, H], F32, kind="ExternalInput").ap()
    bfv = nc.dram_tensor("bfv", [1, H], F32, kind="ExternalInput").ap()
    qn = nc.dram_tensor("qn", [1, HD], F32, kind="ExternalInput").ap()
    kn = nc.dram_tensor("kn", [1, HD], F32, kind="ExternalInput").ap()
    ln1 = nc.dram_tensor("ln1", [1, D], F32, kind="ExternalInput").ap()
    ln2 = nc.dram_tensor("ln2", [1, D], F32, kind="ExternalInput").ap()
    w1 = nc.dram_tensor("w1", [D, FFP], BF16, kind="ExternalInput").ap()
    w2 = nc.dram_tensor("w2", [D, FFP], BF16, kind="ExternalInput").ap()
    w3 = nc.dram_tensor("w3", [FFP, D], BF16, kind="ExternalInput").ap()
    uc = nc.dram_tensor("uc", [S, S], F32, kind="ExternalInput").ap()
    out = nc.dram_tensor("out", [SQ, D], F32, kind="ExternalOutput").ap()

    # [ki, ko, n] views of the d_model-contracting weights
    wq3 = wq.rearrange("(ko ki) n -> ki ko n", ki=P)
    wk3 = wk.rearrange("(ko ki) n -> ki ko n", ki=P)
    wv3 = wv.rearrange("(ko ki) n -> ki ko n", ki=P)
    wo3 = wo.rearrange("(ko ki) n -> ki ko n", ki=P)
    wf3 = wf.rearrange("(ko ki) n -> ki ko n", ki=P)
    w13 = w1.rearrange("(ko ki) n -> ki ko n", ki=P)
    w23 = w2.rearrange("(ko ki) n -> ki ko n", ki=P)

    AT = mybir.ActivationFunctionType
    OP = mybir.AluOpType

    with tile.TileContext(nc) as tc:
        with tc.tile_pool(name="const", bufs=1) as const, \
             tc.tile_pool(name="psmall", bufs=1) as psmall, \
             tc.tile_pool(name="dram", bufs=1, space="DRAM") as dpool:

            ident = const.tile([P, P], F32)
            nc.gpsimd.memset(ident[:], 0.0)
            nc.gpsimd.affine_select(
                out=ident[:], in_=ident[:],
                compare_op=OP.not_equal, fill=1.0,
                base=0, pattern=[[-1, P]], channel_multiplier=1)

            qnb = const.tile([P, HD], F32)
            knb = const.tile([P, HD], F32)
            bfb = const.tile([P, H], F32)
            mtile = const.tile([P, 256], F32)
            nc.sync.dma_start(mtile[:], msk)
            nc.sync.dma_start(qnb[:1, :], qn)
            nc.sync.dma_start(knb[:1, :], kn)
            nc.sync.dma_start(bfb[:1, :], bfv)
            nc.gpsimd.partition_broadcast(qnb[:], qnb[:1, :])
            nc.gpsimd.partition_broadcast(knb[:], knb[:1, :])
            nc.gpsimd.partition_broadcast(bfb[:], bfb[:1, :])

            spn = psmall.tile([P, NBLK, H], F32)   # softplus(-(h@wf+bf))
            csp = psmall.tile([H, S], F32)         # cumsum of spn over tokens

            csp_d = dpool.tile([H, S], F32)         # cumsum staging for bcast
            kT_d = dpool.tile([H, HD, S], BF16)     # k^T per head
            qT_d = dpool.tile([H, HD, SQ], BF16)    # q^T per head
            v_d = dpool.tile([NBLK, P, D], BF16)    # v natural
            x2_d = dpool.tile([SQ, D], F32)        # post-attn residual

            # ---------------- Phase A+B: norms, projections ------------
            def norm_transpose(io, tp, src_ap, dst, nblocks, lnb, tag,
                               logf_fn=None):
                for blk in range(nblocks):
                    xt = io.tile([P, D], F32, tag=f"xt{tag}")
                    nc.sync.dma_start(xt[:], src_ap[blk * P:(blk + 1) * P, :])
                    ssq = io.tile([P, 1], F32, tag=f"ssq{tag}")
                    hh = io.tile([P, D], F32, tag=f"hh{tag}")
                    # square+reduce on DVE (ACT Square would thrash the
                    # activation tables against Ln/Exp)
                    nc.vector.tensor_tensor(hh[:], xt[:], xt[:], OP.mult)
                    nc.vector.reduce_sum(ssq[:], hh[:],
                                         axis=mybir.AxisListType.X)
                    inv = io.tile([P, 1], F32, tag=f"inv{tag}")
                    nc.vector.tensor_scalar(inv[:], ssq[:], 1.0 / D, EPS,
                                            OP.mult, OP.add)
                    nc.scalar.activation(inv[:], inv[:], AT.Sqrt)
                    nc.vector.reciprocal(inv[:], inv[:])
                    nc.vector.tensor_tensor(hh[:], xt[:], lnb[:], OP.mult)
                    nc.vector.tensor_scalar_mul(hh[:], hh[:], inv[:])
                    hf = None
                    if logf_fn is not None:
                        hf = io.tile([P, NDK, P], F32, tag=f"hf{tag}")
                    for kc in range(NDK):
                        pst = tp.tile([P, P], F32, tag="pstn")
                        _transpose(nc, pst[:], hh[:, kc * P:(kc + 1) * P], ident)
                        nc.any.tensor_copy(
                            out=dst[:, kc, blk * P:(blk + 1) * P], in_=pst[:])
                        if hf is not None:
                            nc.any.tensor_copy(out=hf[:, kc, :], in_=pst[:])
                    if logf_fn is not None:
                        logf_fn(blk, hf)

            with tc.tile_pool(name="lnp", bufs=1) as lnp, \
                 tc.tile_pool(name="hpool", bufs=1) as hpool:
                ln1b = lnp.tile([P, D], F32)
                nc.sync.dma_start(ln1b[:1, :], ln1)
                nc.gpsimd.partition_broadcast(ln1b[:], ln1b[:1, :])

                hT = hpool.tile([P, NDK, S], BF16)
                hTq = hpool.tile([P, NDK, SQ], BF16)

                wft = lnp.tile([P, NDK, H], F32)
                nc.sync.dma_start(wft[:], wf3)

                with tc.tile_pool(name="ioA", bufs=2) as io, \
                     tc.tile_pool(name="tpA", bufs=4, space="PSUM") as tp, \
                     tc.tile_pool(name="psL", bufs=2, space="PSUM") as psl:
                    def logf_fn(blk, hf):
                        # forget gate from the fp32 staging of h^T: the
                        # decay path must stay fp32 (errors integrate over
                        # ~1000 cumsum steps before exp()).
                        pl = psl.tile([P, H], F32, tag="psl")
                        for dk in range(NDK):
                            _mm(nc, pl[:], hf[:, dk, :], wft[:, dk, :],
                                start=(dk == 0), stop=(dk == NDK - 1),
                                fast=False)
                        nc.vector.tensor_tensor(spn[:, blk, :], pl[:],
                                                bfb[:], OP.add)

                    norm_transpose(io, tp, xb, hT, NBLK, ln1b, "a",
                                   logf_fn=logf_fn)
                    norm_transpose(io, tp, xq, hTq, NQB, ln1b, "b")
                    # batched softplus(-z) = ln(1 + exp(-z)) over all blocks:
                    # exactly one Exp->Ln table round-trip for the whole phase
                    nc.scalar.activation(spn[:], spn[:], AT.Exp, scale=-1.0)
                    nc.scalar.activation(spn[:], spn[:], AT.Ln, bias=1.0)

                with tc.tile_pool(name="ioB", bufs=3) as io, \
                     tc.tile_pool(name="wB", bufs=2) as wpool, \
                     tc.tile_pool(name="psB", bufs=4, space="PSUM") as ps, \
                     tc.tile_pool(name="tpB", bufs=4, space="PSUM") as tp:

                    # cumulative decay: csp[h, i] = sum_{j<=i} spn[j, h]
                    for it in range(2):
                        pc = ps.tile([P, 512], F32, tag="psb")
                        for blk in range(NBLK):
                            ut = io.tile([P, 512], F32, tag="ut")
                            nc.sync.dma_start(
                                ut[:], uc[blk * P:(blk + 1) * P,
                                          it * 512:(it + 1) * 512])
                            _mm(nc, pc[:H, :], spn[:, blk, :], ut[:],
                                start=(blk == 0), stop=(blk == NBLK - 1),
                                fast=False)
                        nc.any.tensor_copy(out=csp[:, it * 512:(it + 1) * 512],
                                           in_=pc[:H, :])
                    nc.sync.dma_start(csp_d[:], csp[:])

                    # q/k/v projections, 512-wide dout tiles (4 heads each)
                    def qk_proj(w3ap, hsrc, nblocks, nwb, dst_dram):
                        for dt in range(4):
                            wt = wpool.tile([P, NDK, 512], BF16, tag="wt")
                            nc.sync.dma_start(
                                wt[:], w3ap[:, :, dt * 512:(dt + 1) * 512])
                            for blk in range(nblocks):
                                pp = ps.tile([P, 512], F32, tag="psb")
                                for dk in range(NDK):
                                    _mm(nc, pp[:],
                                        hsrc[:, dk, blk * P:(blk + 1) * P],
                                        wt[:, dk, :], start=(dk == 0),
                                        stop=(dk == NDK - 1))
                                for hh in range(4):
                                    head = dt * 4 + hh
                                    hs = slice(hh * HD, (hh + 1) * HD)
                                    qs = io.tile([P, HD], F32, tag="qs")
                                    nc.any.tensor_copy(out=qs[:], in_=pp[:, hs])
                                    sqt = io.tile([P, HD], F32, tag="sqt")
                                    ssq = io.tile([P, 1], F32, tag="ssqh")
                                    nc.vector.tensor_tensor(sqt[:], qs[:],
                                                            qs[:], OP.mult)
                                    nc.vector.reduce_sum(
                                        ssq[:], sqt[:],
                                        axis=mybir.AxisListType.X)
                                    inv = io.tile([P, 1], F32, tag="invh")
                                    nc.vector.tensor_scalar(
                                        inv[:], ssq[:], 1.0 / HD, EPS,
                                        OP.mult, OP.add)
                                    nc.scalar.activation(inv[:], inv[:],
                                                         AT.Sqrt)
                                    nc.vector.reciprocal(inv[:], inv[:])
                                    nc.vector.tensor_scalar_mul(qs[:], qs[:],
                                                                inv[:])
                                    nc.vector.tensor_tensor(qs[:], qs[:],
                                                            nwb[:], OP.mult)
                                    pst = tp.tile([P, P], F32, tag="pstb")
                                    _transpose(nc, pst[:], qs[:], ident)
                                    kst = io.tile([P, P], BF16, tag="kst")
                                    nc.any.tensor_copy(out=kst[:], in_=pst[:])
                                    nc.sync.dma_start(
                                        dst_dram[head, :,
                                                 blk * P:(blk + 1) * P],
                                        kst[:])

                    qk_proj(wq3, hTq, NQB, qnb, qT_d)
                    qk_proj(wk3, hT, NBLK, knb, kT_d)

                    for dt in range(4):
                        wt = wpool.tile([P, NDK, 512], BF16, tag="wt")
                        nc.sync.dma_start(wt[:],
                                          wv3[:, :, dt * 512:(dt + 1) * 512])
                        for blk in range(NBLK):
                            pp = ps.tile([P, 512], F32, tag="psb")
                            for dk in range(NDK):
                                _mm(nc, pp[:], hT[:, dk, blk * P:(blk + 1) * P],
                                    wt[:, dk, :], start=(dk == 0),
                                    stop=(dk == NDK - 1))
                            vs = io.tile([P, 512], BF16, tag="vs")
                            nc.any.tensor_copy(out=vs[:], in_=pp[:])
                            nc.sync.dma_start(
                                v_d[blk, :, dt * 512:(dt + 1) * 512], vs[:])

            # ---------------- Phase C+D: attention + out-proj ----------
            with tc.tile_pool(name="opool", bufs=1) as opool:
                oT = opool.tile([P, H, SQ], BF16)

                with tc.tile_pool(name="ioC", bufs=2) as io, \
                     tc.tile_pool(name="sC", bufs=3) as sc, \
                     tc.tile_pool(name="ptC", bufs=2) as ptp, \
                     tc.tile_pool(name="psC", bufs=3, space="PSUM") as ps, \
                     tc.tile_pool(name="tpC", bufs=2, space="PSUM") as tp, \
                     tc.tile_pool(name="opC", bufs=2, space="PSUM") as op:
                    for head in range(H):
                        crep = io.tile([P, S], F32, tag="crep")
                        nc.sync.dma_start(
                            crep[:], csp_d[head:head + 1, :].broadcast_to((P, S)))
                        kth = io.tile([P, S], BF16, tag="kth")
                        nc.sync.dma_start(kth[:], kT_d[head])
                        qth = io.tile([P, SQ], BF16, tag="qth")
                        nc.sync.dma_start(qth[:], qT_d[head])
                        vh = io.tile([P, NBLK, HD], BF16, tag="vh")
                        nc.sync.dma_start(
                            vh[:], v_d[:, :, head * HD:(head + 1) * HD]
                            .rearrange("b p f -> p b f"))
                        ptb = ptp.tile([P, NBLK, SQ], BF16, tag="ptb")

                        for t in range(NQB):
                            nk = 256 * (t + 1)
                            nparts = (nk + 511) // 512
                            sps = []
                            for prt in range(nparts):
                                w = min(512, nk - prt * 512)
                                pt = ps.tile([P, 512], F32, tag="spsum")
                                _mm(nc, pt[:, :w], qth[:, t * P:(t + 1) * P],
                                    kth[:, prt * 512:prt * 512 + w])
                                sps.append((pt, w))
                            sp = sc.tile([P, S], F32, tag="sprime")
                            for prt, (pt, w) in enumerate(sps):
                                nc.vector.tensor_tensor(
                                    sp[:, prt * 512:prt * 512 + w], pt[:, :w],
                                    crep[:, prt * 512:prt * 512 + w], OP.add)
                            nc.vector.tensor_tensor(
                                sp[:, nk - 256:nk], sp[:, nk - 256:nk],
                                mtile[:], OP.add)
                            mx = io.tile([P, 1], F32, tag="mx")
                            nc.vector.reduce_max(mx[:], sp[:, :nk],
                                                 axis=mybir.AxisListType.X)
                            nc.vector.tensor_scalar_mul(mx[:], mx[:], -1.0)
                            pe = sc.tile([P, S], F32, tag="pexp")
                            rs = io.tile([P, 1], F32, tag="rs")
                            nc.scalar.activation(pe[:, :nk], sp[:, :nk],
                                                 AT.Exp, bias=mx[:],
                                                 accum_out=rs[:])
                            nc.vector.reciprocal(rs[:], rs[:])
                            nc.vector.tensor_scalar_mul(pe[:, :nk],
                                                        pe[:, :nk], rs[:])
                            for kc in range(2 * t + 2):
                                pst = tp.tile([P, P], F32, tag="pstc")
                                _transpose(nc, pst[:],
                                           pe[:, kc * P:(kc + 1) * P], ident)
                                nc.any.tensor_copy(
                                    out=ptb[:, kc, t * P:(t + 1) * P],
                                    in_=pst[:])

                        # Suffix accumulation: key-chunk kc only feeds query
                        # slots t >= kc//2. kc=0 (start=True) clears the whole
                        # bank and covers every column, so unwritten regions
                        # of later chunks are never read.
                        po = op.tile([P, SQ], F32, tag="opsum")
                        for kc in range(NBLK):
                            qlo = (kc // 2) * P
                            _mm(nc, po[:, qlo:], vh[:, kc, :], ptb[:, kc, qlo:],
                                start=(kc == 0), stop=(kc == NBLK - 1))
                        nc.any.tensor_copy(out=oT[:, head, :], in_=po[:])

                with tc.tile_pool(name="ioD", bufs=3) as io, \
                     tc.tile_pool(name="wD", bufs=2) as wpool, \
                     tc.tile_pool(name="psD", bufs=4, space="PSUM") as ps:
                    for dt in range(4):
                        wt = wpool.tile([P, NDK, 512], BF16, tag="wod")
                        nc.sync.dma_start(wt[:],
                                          wo3[:, :, dt * 512:(dt + 1) * 512])
                        for slot in range(NQB):
                            pp = ps.tile([P, 512], F32, tag="psd")
                            for hh in range(H):
                                _mm(nc, pp[:],
                                    oT[:, hh, slot * P:(slot + 1) * P],
                                    wt[:, hh, :], start=(hh == 0),
                                    stop=(hh == H - 1))
                            xqt = io.tile([P, 512], F32, tag="xqt")
                            nc.sync.dma_start(
                                xqt[:], xq[slot * P:(slot + 1) * P,
                                           dt * 512:(dt + 1) * 512])
                            x2t = io.tile([P, 512], F32, tag="x2t")
                            nc.vector.tensor_tensor(x2t[:], pp[:], xqt[:],
                                                    OP.add)
                            nc.sync.dma_start(
                                x2_d[slot * P:(slot + 1) * P,
                                     dt * 512:(dt + 1) * 512], x2t[:])

            # ---------------- Phase E: SwiGLU MLP ----------------------
            with tc.tile_pool(name="mpool", bufs=1) as mpool, \
                 tc.tile_pool(name="ln2p", bufs=1) as ln2p:
                ln2b = ln2p.tile([P, D], F32)
                nc.sync.dma_start(ln2b[:1, :], ln2)
                nc.gpsimd.partition_broadcast(ln2b[:], ln2b[:1, :])

                h2T = mpool.tile([P, NDK, SQ], BF16)
                mT = mpool.tile([P, NFC, SQ], BF16)

                with tc.tile_pool(name="ioE", bufs=2) as io, \
                     tc.tile_pool(name="tpE", bufs=4, space="PSUM") as tp:
                    norm_transpose(io, tp, x2_d, h2T, NQB, ln2b, "e")

                with tc.tile_pool(name="ioF", bufs=3) as io, \
                     tc.tile_pool(name="wE", bufs=2) as wpool, \
                     tc.tile_pool(name="psGU", bufs=2, space="PSUM") as psgu, \
                     tc.tile_pool(name="psY", bufs=1, space="PSUM") as psy:
                    for fc in range(NFC):
                        w1t = wpool.tile([P, NDK, P], BF16, tag="w1t")
                        nc.sync.dma_start(w1t[:], w13[:, :, fc * P:(fc + 1) * P])
                        w2t = wpool.tile([P, NDK, P], BF16, tag="w2t")
                        nc.sync.dma_start(w2t[:], w23[:, :, fc * P:(fc + 1) * P])
                        pg = psgu.tile([P, SQ], F32, tag="pge")
                        for dk in range(NDK):
                            _mm(nc, pg[:], w1t[:, dk, :], h2T[:, dk, :],
                                start=(dk == 0), stop=(dk == NDK - 1))
                        pu = psgu.tile([P, SQ], F32, tag="pue")
                        for dk in range(NDK):
                            _mm(nc, pu[:], w2t[:, dk, :], h2T[:, dk, :],
                                start=(dk == 0), stop=(dk == NDK - 1))
                        gs = io.tile([P, SQ], F32, tag="gse")
                        nc.scalar.activation(gs[:], pg[:], AT.Silu)
                        nc.vector.tensor_tensor(mT[:, fc, :], gs[:], pu[:],
                                                OP.mult)

                    for dt in range(4):
                        yps = [psy.tile([P, 512], F32, tag=f"ype{slot}",
                                        name=f"yps{dt}_{slot}")
                               for slot in range(NQB)]
                        for fc in range(NFC):
                            w3t = wpool.tile([P, 512], BF16, tag="w3t")
                            nc.sync.dma_start(
                                w3t[:], w3[fc * P:(fc + 1) * P,
                                           dt * 512:(dt + 1) * 512])
                            for slot in range(NQB):
                                _mm(nc, yps[slot][:],
                                    mT[:, fc, slot * P:(slot + 1) * P], w3t[:],
                                    start=(fc == 0), stop=(fc == NFC - 1))
                        for slot in range(NQB):
                            x2t = io.tile([P, 512], F32, tag="x2r")
                            nc.sync.dma_start(
                                x2t[:], x2_d[slot * P:(slot + 1) * P,
                                             dt * 512:(dt + 1) * 512])
                            ot = io.tile([P, 512], F32, tag="ote")
                            nc.vector.tensor_tensor(ot[:], yps[slot][:],
                                                    x2t[:], OP.add)
                            nc.sync.dma_start(
                                out[slot * P:(slot + 1) * P,
                                    dt * 512:(dt + 1) * 512], ot[:])

    nc.compile()
    return nc


def _get_nc():
    global _NC
    if _NC is None:
        _NC = build_nc()
    return _NC


def _host_prep(inputs):
    import ml_dtypes
    bf = ml_dtypes.bfloat16
    x = np.ascontiguousarray(np.asarray(inputs["x"], dtype=np.float32))
    w1p = np.zeros((D, FFP), bf)
    w1p[:, :FF] = np.asarray(inputs["w1"]).astype(bf)
    w2p = np.zeros((D, FFP), bf)
    w2p[:, :FF] = np.asarray(inputs["w2"]).astype(bf)
    w3p = np.zeros((FFP, D), bf)
    w3p[:FF, :] = np.asarray(inputs["w3"]).astype(bf)
    uconst = np.triu(np.ones((S, S), np.float32))
    masks = []
    for p in range(2):
        r = np.arange(P)[:, None]
        f = np.arange(256)[None, :]
        masks.append(np.where(f <= 2 * r + p, 0.0, NEG).astype(np.float32))
    shared = {
        "wq": np.asarray(inputs["wq"]).astype(bf),
        "wk": np.asarray(inputs["wk"]).astype(bf),
        "wv": np.asarray(inputs["wv"]).astype(bf),
        "wo": np.asarray(inputs["wo"]).astype(bf),
        "wf": np.ascontiguousarray(inputs["wf"], dtype=np.float32),
        "bfv": np.asarray(inputs["bf"], dtype=np.float32).reshape(1, H),
        "qn": (np.asarray(inputs["qn_w"], dtype=np.float32)
               / np.sqrt(HD).astype(np.float32)).reshape(1, HD),
        "kn": np.asarray(inputs["kn_w"], dtype=np.float32).reshape(1, HD),
        "ln1": np.asarray(inputs["ln1_w"], dtype=np.float32).reshape(1, D),
        "ln2": np.asarray(inputs["ln2_w"], dtype=np.float32).reshape(1, D),
        "w1": w1p, "w2": w2p, "w3": w3p, "uc": uconst,
    }
    in_maps = []
    for c in range(8):
        b, p = c // 2, c % 2
        m = dict(shared)
        m["xb"] = np.ascontiguousarray(x[b])
        m["xq"] = np.ascontiguousarray(x[b][p::2])
        m["msk"] = masks[p]
        in_maps.append(m)
    return in_maps


def run(inputs, trace=False, tmpdir=None):
    nc = _get_nc()
    in_maps = _host_prep(inputs)
    res = bass_utils.run_bass_kernel_spmd(
        nc, in_maps, core_ids=list(range(8)), trace=trace, tmpdir=tmpdir)
    out = np.empty((B, S, D), np.float32)
    for c in range(8):
        b, p = c // 2, c % 2
        out[b, p::2, :] = res.results[c]["out"]
    return out, res


def kernel(**inputs):
    out, _ = run(inputs, trace=False)
    return out


# revision 20
# speedup vs baseline: 1.6508x; 1.1030x over previous
"""Trainium2 Bass kernel for nn_Block_64338610094594.

Forgetting-attention transformer block (rmsnorm -> qkv + per-head
data-dependent log-sigmoid forget gate -> decayed causal softmax ->
out-proj + residual -> rmsnorm -> SwiGLU MLP + residual).

Sharding: 8 cores, data-parallel. Core c owns batch b = c//2 and the
query rows of parity p = c%2 (rows p, p+2, ... of that batch). Each
core of a pair computes K/V/logf for its whole batch (redundant), so
attention, out-proj and MLP need no cross-core communication at all.
The fine row-interleave makes the causal-attention work identical on
every core (uniform SPMD program; per-core differences enter only
through data: the xq gather and the causal mask tile).

Matmul strategy: bf16 operands for all large matmuls (full PE rate,
cheap hidden LDWEIGHTS, half the weight DMA), fp32 PSUM accumulation.
The forget-gate projection and the cumulative-sum matmul stay plain
fp32 (their error integrates over ~1000 decay steps before exp()).
"""

import numpy as np

import concourse.bass as bass
import concourse.mybir as mybir
import concourse.tile as tile
from concourse import bacc
from concourse import bass_utils

F32 = mybir.dt.float32
F32R = mybir.dt.float32r
BF16 = mybir.dt.bfloat16

B, S, D = 4, 1024, 2048
H, HD = 16, 128
FF = 5461
FFP = 5504          # FF padded to 43*128
NFC = FFP // 128    # 43 ff chunks
EPS = 1e-5
P = 128
NBLK = S // P       # 8 token blocks per batch
SQ = S // 2         # 512 query rows per core
NQB = SQ // P       # 4 query slots
NDK = D // P        # 16 contraction chunks of d_model
NEG = -1.0e30

_NC = None


def _f32r(ap):
    return ap.bitcast(F32R)


def _mm(nc, out, lhsT, rhs, start=True, stop=True, fast=True):
    nc.tensor.matmul(out, lhsT, rhs, start=start, stop=stop)


def _transpose(nc, out_psum, in_sbuf, ident):
    nc.tensor.transpose(out_psum, in_sbuf, ident)


def build_nc():
    nc = bacc.Bacc("TRN2", target_bir_lowering=False, debug=False)

    xb = nc.dram_tensor("xb", [S, D], F32, kind="ExternalInput").ap()
    xq = nc.dram_tensor("xq", [SQ, D], F32, kind="ExternalInput").ap()
    msk = nc.dram_tensor("msk", [P, 256], F32, kind="ExternalInput").ap()
    wq = nc.dram_tensor("wq", [D, D], BF16, kind="ExternalInput").ap()
    wk = nc.dram_tensor("wk", [D, D], BF16, kind="ExternalInput").ap()
    wv = nc.dram_tensor("wv", [D, D], BF16, kind="ExternalInput").ap()
    wo = nc.dram_tensor("wo", [D, D], BF16, kind="ExternalInput").ap()
    wf = nc.dram_tensor("wf", [D, H], F32, kind="ExternalInput").ap()
    bfv = nc.dram_tensor("bfv", [1, H], F32, kind="ExternalInput").ap()
    qn = nc.dram_tensor("qn", [1, HD], F32, kind="ExternalInput").ap()
    kn = nc.dram_tensor("kn", [1, HD], F32, kind="ExternalInput").ap()
    ln1 = nc.dram_tensor("ln1", [1, D], F32, kind="ExternalInput").ap()
    ln2 = nc.dram_tensor("ln2", [1, D], F32, kind="ExternalInput").ap()
    w1 = nc.dram_tensor("w1", [D, FFP], BF16, kind="ExternalInput").ap()
    w2 = nc.dram_tensor("w2", [D, FFP], BF16, kind="ExternalInput").ap()
    w3 = nc.dram_tensor("w3", [FFP, D], BF16, kind="ExternalInput").ap()
    uc = nc.dram_tensor("uc", [S, S], F32, kind="ExternalInput").ap()
    out = nc.dram_tensor("out", [SQ, D], F32, kind="ExternalOutput").ap()

    # [ki, ko, n] views of the d_model-contracting weights
    wq3 = wq.rearrange("(ko ki) n -> ki ko n", ki=P)
    wk3 = wk.rearrange("(ko ki) n -> ki ko n", ki=P)
    wv3 = wv.rearrange("(ko ki) n -> ki ko n", ki=P)
    wo3 = wo.rearrange("(ko ki) n -> ki ko n", ki=P)
    wf3 = wf.rearrange("(ko ki) n -> ki ko n", ki=P)
    w13 = w1.rearrange("(ko ki) n -> ki ko n", ki=P)
    w23 = w2.rearrange("(ko ki) n -> ki ko n", ki=P)

    AT = mybir.ActivationFunctionType
    OP = mybir.AluOpType

    with tile.TileContext(nc) as tc:
        with tc.tile_pool(name="const", bufs=1) as const, \
             tc.tile_pool(name="psmall", bufs=1) as psmall, \
             tc.tile_pool(name="dram", bufs=1, space="DRAM") as dpool:

            ident = const.tile([P, P], F32)
            nc.gpsimd.memset(ident[:], 0.0)
            nc.gpsimd.affine_select(
                out=ident[:], in_=ident[:],
                compare_op=OP.not_equal, fill=1.0,
                base=0, pattern=[[-1, P]], channel_multiplier=1)

            qnb = const.tile([P, HD], F32)
            knb = const.tile([P, HD], F32)
            bfb = const.tile([P, H], F32)
            mtile = const.tile([P, 256], F32)
            nc.sync.dma_start(mtile[:], msk)
            nc.sync.dma_start(qnb[:1, :], qn)
            nc.sync.dma_start(knb[:1, :], kn)
            nc.sync.dma_start(bfb[:1, :], bfv)
            nc.gpsimd.partition_broadcast(qnb[:], qnb[:1, :])
            nc.gpsimd.partition_broadcast(knb[:], knb[:1, :])
            nc.gpsimd.partition_broadcast(bfb[:], bfb[:1, :])

            spn = psmall.tile([P, NBLK, H], F32)   # softplus(-(h@wf+bf))
            csp = psmall.tile([H, S], F32)         # cumsum of spn over tokens

            csp_d = dpool.tile([H, S], F32)         # cumsum staging for bcast
            kT_d = dpool.tile([H, HD, S], BF16)     # k^T per head
            qT_d = dpool.tile([H, HD, SQ], BF16)    # q^T per head
            v_d = dpool.tile([NBLK, P, D], BF16)    # v natural
            x2_d = dpool.tile([SQ, D], F32)        # post-attn residual

            # ---------------- Phase A+B: norms, projections ------------
            def norm_transpose(io, tp, src_ap, dst, nblocks, lnb, tag,
                               logf_fn=None):
                for blk in range(nblocks):
                    xt = io.tile([P, D], F32, tag=f"xt{tag}")
                    nc.sync.dma_start(xt[:], src_ap[blk * P:(blk + 1) * P, :])
                    ssq = io.tile([P, 1], F32, tag=f"ssq{tag}")
                    hh = io.tile([P, D], F32, tag=f"hh{tag}")
                    # square+reduce on DVE (ACT Square would thrash the
                    # activation tables against Ln/Exp)
                    nc.vector.tensor_tensor(hh[:], xt[:], xt[:], OP.mult)
                    nc.vector.reduce_sum(ssq[:], hh[:],
                                         axis=mybir.AxisListType.X)
                    inv = io.tile([P, 1], F32, tag=f"inv{tag}")
                    nc.vector.tensor_scalar(inv[:], ssq[:], 1.0 / D, EPS,
                                            OP.mult, OP.add)
                    nc.scalar.activation(inv[:], inv[:], AT.Sqrt)
                    nc.vector.reciprocal(inv[:], inv[:])
                    nc.vector.tensor_tensor(hh[:], xt[:], lnb[:], OP.mult)
                    nc.vector.tensor_scalar_mul(hh[:], hh[:], inv[:])
                    hf = None
                    if logf_fn is not None:
                        hf = io.tile([P, NDK, P], F32, tag=f"hf{tag}")
                    for kc in range(NDK):
                        pst = tp.tile([P, P], F32, tag="pstn")
                        _transpose(nc, pst[:], hh[:, kc * P:(kc + 1) * P], ident)
                        nc.any.tensor_copy(
                            out=dst[:, kc, blk * P:(blk + 1) * P], in_=pst[:])
                        if hf is not None:
                            nc.any.tensor_copy(out=hf[:, kc, :], in_=pst[:])
                    if logf_fn is not None:
                        logf_fn(blk, hf)

            with tc.tile_pool(name="lnp", bufs=1) as lnp, \
                 tc.tile_pool(name="hpool", bufs=1) as hpool, \
                 tc.tile_pool(name="wB", bufs=2) as wpoolB:
                ln1b = lnp.tile([P, D], F32)
                nc.sync.dma_start(ln1b[:1, :], ln1)
                nc.gpsimd.partition_broadcast(ln1b[:], ln1b[:1, :])

                hT = hpool.tile([P, NDK, S], BF16)
                hTq = hpool.tile([P, NDK, SQ], BF16)

                wft = lnp.tile([P, NDK, H], F32)
                nc.sync.dma_start(wft[:], wf3)

                with tc.tile_pool(name="ioA", bufs=2) as io, \
                     tc.tile_pool(name="tpA", bufs=4, space="PSUM") as tp, \
                     tc.tile_pool(name="psL", bufs=2, space="PSUM") as psl:
                    def logf_fn(blk, hf):
                        # forget gate from the fp32 staging of h^T: the
                        # decay path must stay fp32 (errors integrate over
                        # ~1000 cumsum steps before exp()).
                        pl = psl.tile([P, H], F32, tag="psl")
                        for dk in range(NDK):
                            _mm(nc, pl[:], hf[:, dk, :], wft[:, dk, :],
                                start=(dk == 0), stop=(dk == NDK - 1),
                                fast=False)
                        nc.vector.tensor_tensor(spn[:, blk, :], pl[:],
                                                bfb[:], OP.add)

                    norm_transpose(io, tp, xb, hT, NBLK, ln1b, "a",
                                   logf_fn=logf_fn)
                    norm_transpose(io, tp, xq, hTq, NQB, ln1b, "b")
                    # batched softplus(-z) = ln(1 + exp(-z)) over all blocks:
                    # exactly one Exp->Ln table round-trip for the whole phase
                    nc.scalar.activation(spn[:], spn[:], AT.Exp, scale=-1.0)
                    nc.scalar.activation(spn[:], spn[:], AT.Ln, bias=1.0)

                with tc.tile_pool(name="ioB", bufs=3) as io, \
                     tc.tile_pool(name="psB", bufs=4, space="PSUM") as ps, \
                     tc.tile_pool(name="tpB", bufs=4, space="PSUM") as tp:
                    wpool = wpoolB

                    # cumulative decay: csp[h, i] = sum_{j<=i} spn[j, h]
                    for it in range(2):
                        pc = ps.tile([P, 512], F32, tag="psb")
                        for blk in range(NBLK):
                            ut = io.tile([P, 512], F32, tag="ut")
                            nc.sync.dma_start(
                                ut[:], uc[blk * P:(blk + 1) * P,
                                          it * 512:(it + 1) * 512])
                            _mm(nc, pc[:H, :], spn[:, blk, :], ut[:],
                                start=(blk == 0), stop=(blk == NBLK - 1),
                                fast=False)
                        nc.any.tensor_copy(out=csp[:, it * 512:(it + 1) * 512],
                                           in_=pc[:H, :])
                    nc.sync.dma_start(csp_d[:], csp[:])

                    # q/k/v projections, 512-wide dout tiles (4 heads each)
                    def qk_proj(w3ap, hsrc, nblocks, nwb, dst_dram):
                        for dt in range(4):
                            wt = wpool.tile([P, NDK, 512], BF16, tag="wt")
                            nc.sync.dma_start(
                                wt[:], w3ap[:, :, dt * 512:(dt + 1) * 512])
                            for blk in range(nblocks):
                                pp = ps.tile([P, 512], F32, tag="psb")
                                for dk in range(NDK):
                                    _mm(nc, pp[:],
                                        hsrc[:, dk, blk * P:(blk + 1) * P],
                                        wt[:, dk, :], start=(dk == 0),
                                        stop=(dk == NDK - 1))
                                for hh in range(4):
                                    head = dt * 4 + hh
                                    hs = slice(hh * HD, (hh + 1) * HD)
                                    qs = io.tile([P, HD], F32, tag="qs")
                                    nc.any.tensor_copy(out=qs[:], in_=pp[:, hs])
                                    sqt = io.tile([P, HD], F32, tag="sqt")
                                    ssq = io.tile([P, 1], F32, tag="ssqh")
                                    nc.vector.tensor_tensor(sqt[:], qs[:],
                                                            qs[:], OP.mult)
                                    nc.vector.reduce_sum(
                                        ssq[:], sqt[:],
                                        axis=mybir.AxisListType.X)
                                    inv = io.tile([P, 1], F32, tag="invh")
                                    nc.vector.tensor_scalar(
                                        inv[:], ssq[:], 1.0 / HD, EPS,
                                        OP.mult, OP.add)
                                    nc.scalar.activation(inv[:], inv[:],
                                                         AT.Sqrt)
                                    nc.vector.reciprocal(inv[:], inv[:])
                                    nc.vector.tensor_scalar_mul(qs[:], qs[:],
                                                                inv[:])
                                    nc.vector.tensor_tensor(qs[:], qs[:],
                                                            nwb[:], OP.mult)
                                    pst = tp.tile([P, P], F32, tag="pstb")
                                    _transpose(nc, pst[:], qs[:], ident)
                                    kst = io.tile([P, P], BF16, tag="kst")
                                    nc.any.tensor_copy(out=kst[:], in_=pst[:])
                                    nc.sync.dma_start(
                                        dst_dram[head, :,
                                                 blk * P:(blk + 1) * P],
                                        kst[:])

                    qk_proj(wq3, hTq, NQB, qnb, qT_d)
                    qk_proj(wk3, hT, NBLK, knb, kT_d)

                    for dt in range(4):
                        wt = wpool.tile([P, NDK, 512], BF16, tag="wt")
                        nc.sync.dma_start(wt[:],
                                          wv3[:, :, dt * 512:(dt + 1) * 512])
                        for blk in range(NBLK):
                            pp = ps.tile([P, 512], F32, tag="psb")
                            for dk in range(NDK):
                                _mm(nc, pp[:], hT[:, dk, blk * P:(blk + 1) * P],
                                    wt[:, dk, :], start=(dk == 0),
                                    stop=(dk == NDK - 1))
                            vs = io.tile([P, 512], BF16, tag="vs")
                            nc.any.tensor_copy(out=vs[:], in_=pp[:])
                            nc.sync.dma_start(
                                v_d[blk, :, dt * 512:(dt + 1) * 512], vs[:])

            # ---------------- Phase C+D: attention + out-proj ----------
            with tc.tile_pool(name="opool", bufs=1) as opool, \
                 tc.tile_pool(name="wD", bufs=2) as wpoolD:
                oT = opool.tile([P, H, SQ], BF16)

                with tc.tile_pool(name="ioC", bufs=2) as io, \
                     tc.tile_pool(name="sC", bufs=3) as sc, \
                     tc.tile_pool(name="ptC", bufs=2) as ptp, \
                     tc.tile_pool(name="psC", bufs=3, space="PSUM") as ps, \
                     tc.tile_pool(name="tpC", bufs=2, space="PSUM") as tp, \
                     tc.tile_pool(name="opC", bufs=2, space="PSUM") as op:
                    for head in range(H):
                        crep = io.tile([P, S], F32, tag="crep")
                        nc.sync.dma_start(
                            crep[:], csp_d[head:head + 1, :].broadcast_to((P, S)))
                        kth = io.tile([P, S], BF16, tag="kth")
                        nc.sync.dma_start(kth[:], kT_d[head])
                        qth = io.tile([P, SQ], BF16, tag="qth")
                        nc.sync.dma_start(qth[:], qT_d[head])
                        vh = io.tile([P, NBLK, HD], BF16, tag="vh")
                        nc.sync.dma_start(
                            vh[:], v_d[:, :, head * HD:(head + 1) * HD]
                            .rearrange("b p f -> p b f"))
                        ptb = ptp.tile([P, NBLK, SQ], BF16, tag="ptb")

                        for t in range(NQB):
                            nk = 256 * (t + 1)
                            nparts = (nk + 511) // 512
                            sps = []
                            for prt in range(nparts):
                                w = min(512, nk - prt * 512)
                                pt = ps.tile([P, 512], F32, tag="spsum")
                                _mm(nc, pt[:, :w], qth[:, t * P:(t + 1) * P],
                                    kth[:, prt * 512:prt * 512 + w])
                                sps.append((pt, w))
                            sp = sc.tile([P, S], F32, tag="sprime")
                            for prt, (pt, w) in enumerate(sps):
                                nc.vector.tensor_tensor(
                                    sp[:, prt * 512:prt * 512 + w], pt[:, :w],
                                    crep[:, prt * 512:prt * 512 + w], OP.add)
                            nc.vector.tensor_tensor(
                                sp[:, nk - 256:nk], sp[:, nk - 256:nk],
                                mtile[:], OP.add)
                            mx = io.tile([P, 1], F32, tag="mx")
                            nc.vector.reduce_max(mx[:], sp[:, :nk],
                                                 axis=mybir.AxisListType.X)
                            nc.vector.tensor_scalar_mul(mx[:], mx[:], -1.0)
                            pe = sc.tile([P, S], F32, tag="pexp")
                            rs = io.tile([P, 1], F32, tag="rs")
                            nc.scalar.activation(pe[:, :nk], sp[:, :nk],
                                                 AT.Exp, bias=mx[:],
                                                 accum_out=rs[:])
                            nc.vector.reciprocal(rs[:], rs[:])
                            nc.vector.tensor_scalar_mul(pe[:, :nk],
                                                        pe[:, :nk], rs[:])
                            for kc in range(2 * t + 2):
                                pst = tp.tile([P, P], F32, tag="pstc")
                                _transpose(nc, pst[:],
                                           pe[:, kc * P:(kc + 1) * P], ident)
                                nc.any.tensor_copy(
                                    out=ptb[:, kc, t * P:(t + 1) * P],
                                    in_=pst[:])

                        # Suffix accumulation: key-chunk kc only feeds query
                        # slots t >= kc//2. kc=0 (start=True) clears the whole
                        # bank and covers every column, so unwritten regions
                        # of later chunks are never read.
                        po = op.tile([P, SQ], F32, tag="opsum")
                        for kc in range(NBLK):
                            qlo = (kc // 2) * P
                            _mm(nc, po[:, qlo:], vh[:, kc, :], ptb[:, kc, qlo:],
                                start=(kc == 0), stop=(kc == NBLK - 1))
                        nc.any.tensor_copy(out=oT[:, head, :], in_=po[:])

                with tc.tile_pool(name="ioD", bufs=3) as io, \
                     tc.tile_pool(name="psD", bufs=4, space="PSUM") as ps:
                    wpool = wpoolD
                    for dt in range(4):
                        wt = wpool.tile([P, NDK, 512], BF16, tag="wod")
                        nc.sync.dma_start(wt[:],
                                          wo3[:, :, dt * 512:(dt + 1) * 512])
                        for slot in range(NQB):
                            pp = ps.tile([P, 512], F32, tag="psd")
                            for hh in range(H):
                                _mm(nc, pp[:],
                                    oT[:, hh, slot * P:(slot + 1) * P],
                                    wt[:, hh, :], start=(hh == 0),
                                    stop=(hh == H - 1))
                            xqt = io.tile([P, 512], F32, tag="xqt")
                            nc.sync.dma_start(
                                xqt[:], xq[slot * P:(slot + 1) * P,
                                           dt * 512:(dt + 1) * 512])
                            x2t = io.tile([P, 512], F32, tag="x2t")
                            nc.vector.tensor_tensor(x2t[:], pp[:], xqt[:],
                                                    OP.add)
                            nc.sync.dma_start(
                                x2_d[slot * P:(slot + 1) * P,
                                     dt * 512:(dt + 1) * 512], x2t[:])

            # ---------------- Phase E: SwiGLU MLP ----------------------
            with tc.tile_pool(name="mpool", bufs=1) as mpool, \
                 tc.tile_pool(name="ln2p", bufs=1) as ln2p, \
                 tc.tile_pool(name="wE", bufs=3) as wpoolE:
                ln2b = ln2p.tile([P, D], F32)
                nc.sync.dma_start(ln2b[:1, :], ln2)
                nc.gpsimd.partition_broadcast(ln2b[:], ln2b[:1, :])

                h2T = mpool.tile([P, NDK, SQ], BF16)
                mT = mpool.tile([P, NFC, SQ], BF16)

                with tc.tile_pool(name="ioE", bufs=2) as io, \
                     tc.tile_pool(name="tpE", bufs=4, space="PSUM") as tp:
                    norm_transpose(io, tp, x2_d, h2T, NQB, ln2b, "e")

                with tc.tile_pool(name="ioF", bufs=3) as io, \
                     tc.tile_pool(name="psGU", bufs=2, space="PSUM") as psgu, \
                     tc.tile_pool(name="psY", bufs=1, space="PSUM") as psy:
                    wpool = wpoolE
                    for fc in range(NFC):
                        w1t = wpool.tile([P, NDK, P], BF16, tag="w1t")
                        nc.sync.dma_start(w1t[:], w13[:, :, fc * P:(fc + 1) * P])
                        w2t = wpool.tile([P, NDK, P], BF16, tag="w2t")
                        nc.sync.dma_start(w2t[:], w23[:, :, fc * P:(fc + 1) * P])
                        pg = psgu.tile([P, SQ], F32, tag="pge")
                        for dk in range(NDK):
                            _mm(nc, pg[:], w1t[:, dk, :], h2T[:, dk, :],
                                start=(dk == 0), stop=(dk == NDK - 1))
                        pu = psgu.tile([P, SQ], F32, tag="pue")
                        for dk in range(NDK):
                            _mm(nc, pu[:], w2t[:, dk, :], h2T[:, dk, :],
                                start=(dk == 0), stop=(dk == NDK - 1))
                        gs = io.tile([P, SQ], F32, tag="gse")
                        nc.scalar.activation(gs[:], pg[:], AT.Silu)
                        nc.vector.tensor_tensor(mT[:, fc, :], gs[:], pu[:],
                                                OP.mult)

                    for dt in range(4):
                        yps = [psy.tile([P, 512], F32, tag=f"ype{slot}",
                                        name=f"yps{dt}_{slot}")
                               for slot in range(NQB)]
                        for fc in range(NFC):
                            w3t = wpool.tile([P, 512], BF16, tag="w3t")
                            nc.sync.dma_start(
                                w3t[:], w3[fc * P:(fc + 1) * P,
                                           dt * 512:(dt + 1) * 512])
                            for slot in range(NQB):
                                _mm(nc, yps[slot][:],
                                    mT[:, fc, slot * P:(slot + 1) * P], w3t[:],
                                    start=(fc == 0), stop=(fc == NFC - 1))
                        for slot in range(NQB):
                            x2t = io.tile([P, 512], F32, tag="x2r")
                            nc.sync.dma_start(
                                x2t[:], x2_d[slot * P:(slot + 1) * P,
                                             dt * 512:(dt + 1) * 512])
                            ot = io.tile([P, 512], F32, tag="ote")
                            nc.vector.tensor_tensor(ot[:], yps[slot][:],
                                                    x2t[:], OP.add)
                            nc.sync.dma_start(
                                out[slot * P:(slot + 1) * P,
                                    dt * 512:(dt + 1) * 512], ot[:])

    nc.compile()
    return nc


def _get_nc():
    global _NC
    if _NC is None:
        _NC = build_nc()
    return _NC


def _host_prep(inputs):
    import ml_dtypes
    bf = ml_dtypes.bfloat16
    x = np.ascontiguousarray(np.asarray(inputs["x"], dtype=np.float32))
    w1p = np.zeros((D, FFP), bf)
    w1p[:, :FF] = np.asarray(inputs["w1"]).astype(bf)
    w2p = np.zeros((D, FFP), bf)
    w2p[:, :FF] = np.asarray(inputs["w2"]).astype(bf)
    w3p = np.zeros((FFP, D), bf)
    w3p[:FF, :] = np.asarray(inputs["w3"]).astype(bf)
    uconst = np.triu(np.ones((S, S), np.float32))
    masks = []
    for p in range(2):
        r = np.arange(P)[:, None]
        f = np.arange(256)[None, :]
        masks.append(np.where(f <= 2 * r + p, 0.0, NEG).astype(np.float32))
    shared = {
        "wq": np.asarray(inputs["wq"]).astype(bf),
        "wk": np.asarray(inputs["wk"]).astype(bf),
        "wv": np.asarray(inputs["wv"]).astype(bf),
        "wo": np.asarray(inputs["wo"]).astype(bf),
        "wf": np.ascontiguousarray(inputs["wf"], dtype=np.float32),
        "bfv": np.asarray(inputs["bf"], dtype=np.float32).reshape(1, H),
        "qn": (np.asarray(inputs["qn_w"], dtype=np.float32)
               / np.sqrt(HD).astype(np.float32)).reshape(1, HD),
        "kn": np.asarray(inputs["kn_w"], dtype=np.float32).reshape(1, HD),
        "ln1": np.asarray(inputs["ln1_w"], dtype=np.float32).reshape(1, D),
        "ln2": np.asarray(inputs["ln2_w"], dtype=np.float32).reshape(1, D),
        "w1": w1p, "w2": w2p, "w3": w3p, "uc": uconst,
    }
    in_maps = []
    for c in range(8):
        b, p = c // 2, c % 2
        m = dict(shared)
        m["xb"] = np.ascontiguousarray(x[b])
        m["xq"] = np.ascontiguousarray(x[b][p::2])
        m["msk"] = masks[p]
        in_maps.append(m)
    return in_maps


def run(inputs, trace=False, tmpdir=None):
    nc = _get_nc()
    in_maps = _host_prep(inputs)
    res = bass_utils.run_bass_kernel_spmd(
        nc, in_maps, core_ids=list(range(8)), trace=trace, tmpdir=tmpdir)
    out = np.empty((B, S, D), np.float32)
    for c in range(8):
        b, p = c // 2, c % 2
        out[b, p::2, :] = res.results[c]["out"]
    return out, res


def kernel(**inputs):
    out, _ = run(inputs, trace=False)
    return out


# revision 22
# speedup vs baseline: 1.7489x; 1.0594x over previous
"""Trainium2 Bass kernel for nn_Block_64338610094594.

Forgetting-attention transformer block (rmsnorm -> qkv + per-head
data-dependent log-sigmoid forget gate -> decayed causal softmax ->
out-proj + residual -> rmsnorm -> SwiGLU MLP + residual).

Sharding: 8 cores, data-parallel. Core c owns batch b = c//2 and the
query rows of parity p = c%2 (rows p, p+2, ... of that batch). Each
core of a pair computes K/V/logf for its whole batch (redundant), so
attention, out-proj and MLP need no cross-core communication at all.
The fine row-interleave makes the causal-attention work identical on
every core (uniform SPMD program; per-core differences enter only
through data: the xq gather and the causal mask tile).

Matmul strategy: bf16 operands for all large matmuls (full PE rate,
cheap hidden LDWEIGHTS, half the weight DMA), fp32 PSUM accumulation.
The forget-gate projection and the cumulative-sum matmul stay plain
fp32 (their error integrates over ~1000 decay steps before exp()).
"""

import numpy as np

import concourse.bass as bass
import concourse.mybir as mybir
import concourse.tile as tile
from concourse import bacc
from concourse import bass_utils

F32 = mybir.dt.float32
F32R = mybir.dt.float32r
BF16 = mybir.dt.bfloat16

B, S, D = 4, 1024, 2048
H, HD = 16, 128
FF = 5461
FFP = 5504          # FF padded to 43*128
NFC = FFP // 128    # 43 ff chunks
EPS = 1e-5
P = 128
NBLK = S // P       # 8 token blocks per batch
SQ = S // 2         # 512 query rows per core
NQB = SQ // P       # 4 query slots
NDK = D // P        # 16 contraction chunks of d_model
NEG = -1.0e30

_NC = None


def _f32r(ap):
    return ap.bitcast(F32R)


def _mm(nc, out, lhsT, rhs, start=True, stop=True, fast=True):
    nc.tensor.matmul(out, lhsT, rhs, start=start, stop=stop)


def _transpose(nc, out_psum, in_sbuf, ident):
    nc.tensor.transpose(out_psum, in_sbuf, ident)


def build_nc():
    nc = bacc.Bacc("TRN2", target_bir_lowering=False, debug=False)

    xb = nc.dram_tensor("xb", [S, D], F32, kind="ExternalInput").ap()
    xq = nc.dram_tensor("xq", [SQ, D], F32, kind="ExternalInput").ap()
    msk = nc.dram_tensor("msk", [P, 256], F32, kind="ExternalInput").ap()
    wq = nc.dram_tensor("wq", [D, D], BF16, kind="ExternalInput").ap()
    wk = nc.dram_tensor("wk", [D, D], BF16, kind="ExternalInput").ap()
    wv = nc.dram_tensor("wv", [D, D], BF16, kind="ExternalInput").ap()
    wo = nc.dram_tensor("wo", [D, D], BF16, kind="ExternalInput").ap()
    wf = nc.dram_tensor("wf", [D, H], F32, kind="ExternalInput").ap()
    bfv = nc.dram_tensor("bfv", [1, H], F32, kind="ExternalInput").ap()
    qn = nc.dram_tensor("qn", [1, HD], F32, kind="ExternalInput").ap()
    kn = nc.dram_tensor("kn", [1, HD], F32, kind="ExternalInput").ap()
    ln1 = nc.dram_tensor("ln1", [1, D], F32, kind="ExternalInput").ap()
    ln2 = nc.dram_tensor("ln2", [1, D], F32, kind="ExternalInput").ap()
    w1 = nc.dram_tensor("w1", [D, FFP], BF16, kind="ExternalInput").ap()
    w2 = nc.dram_tensor("w2", [D, FFP], BF16, kind="ExternalInput").ap()
    w3 = nc.dram_tensor("w3", [FFP, D], BF16, kind="ExternalInput").ap()
    uc = nc.dram_tensor("uc", [S, S], F32, kind="ExternalInput").ap()
    out = nc.dram_tensor("out", [SQ, D], F32, kind="ExternalOutput").ap()

    # [ki, ko, n] views of the d_model-contracting weights
    wq3 = wq.rearrange("(ko ki) n -> ki ko n", ki=P)
    wk3 = wk.rearrange("(ko ki) n -> ki ko n", ki=P)
    wv3 = wv.rearrange("(ko ki) n -> ki ko n", ki=P)
    wo3 = wo.rearrange("(ko ki) n -> ki ko n", ki=P)
    wf3 = wf.rearrange("(ko ki) n -> ki ko n", ki=P)
    w13 = w1.rearrange("(ko ki) n -> ki ko n", ki=P)
    w23 = w2.rearrange("(ko ki) n -> ki ko n", ki=P)

    AT = mybir.ActivationFunctionType
    OP = mybir.AluOpType

    with tile.TileContext(nc) as tc:
        with tc.tile_pool(name="const", bufs=1) as const, \
             tc.tile_pool(name="psmall", bufs=1) as psmall, \
             tc.tile_pool(name="dram", bufs=1, space="DRAM") as dpool:

            ident = const.tile([P, P], F32)
            nc.gpsimd.memset(ident[:], 0.0)
            nc.gpsimd.affine_select(
                out=ident[:], in_=ident[:],
                compare_op=OP.not_equal, fill=1.0,
                base=0, pattern=[[-1, P]], channel_multiplier=1)

            qnb = const.tile([P, 4, HD], F32)
            knb = const.tile([P, 4, HD], F32)
            bfb = const.tile([P, H], F32)
            mtile = const.tile([P, 256], F32)
            nc.sync.dma_start(mtile[:], msk)
            for r in range(4):
                nc.sync.dma_start(qnb[:1, r, :], qn)
                nc.sync.dma_start(knb[:1, r, :], kn)
            nc.sync.dma_start(bfb[:1, :], bfv)
            nc.gpsimd.partition_broadcast(qnb[:], qnb[:1])
            nc.gpsimd.partition_broadcast(knb[:], knb[:1])
            nc.gpsimd.partition_broadcast(bfb[:], bfb[:1, :])

            spn = psmall.tile([P, NBLK, H], F32)   # softplus(-(h@wf+bf))
            csp = psmall.tile([H, S], F32)         # cumsum of spn over tokens

            csp_d = dpool.tile([H, S], F32)         # cumsum staging for bcast
            kT_d = dpool.tile([H, HD, S], BF16)     # k^T per head
            qT_d = dpool.tile([H, HD, SQ], BF16)    # q^T per head
            v_d = dpool.tile([NBLK, P, D], BF16)    # v natural
            x2_d = dpool.tile([SQ, D], F32)        # post-attn residual

            # ---------------- Phase A+B: norms, projections ------------
            def norm_transpose(io, tp, src_ap, dst, nblocks, lnb, tag,
                               logf_fn=None):
                for blk in range(nblocks):
                    xt = io.tile([P, D], F32, tag=f"xt{tag}")
                    nc.sync.dma_start(xt[:], src_ap[blk * P:(blk + 1) * P, :])
                    ssq = io.tile([P, 1], F32, tag=f"ssq{tag}")
                    hh = io.tile([P, D], F32, tag=f"hh{tag}")
                    # square+reduce on DVE (ACT Square would thrash the
                    # activation tables against Ln/Exp)
                    nc.vector.tensor_tensor(hh[:], xt[:], xt[:], OP.mult)
                    nc.vector.reduce_sum(ssq[:], hh[:],
                                         axis=mybir.AxisListType.X)
                    inv = io.tile([P, 1], F32, tag=f"inv{tag}")
                    nc.vector.tensor_scalar(inv[:], ssq[:], 1.0 / D, EPS,
                                            OP.mult, OP.add)
                    nc.scalar.activation(inv[:], inv[:], AT.Sqrt)
                    nc.vector.reciprocal(inv[:], inv[:])
                    nc.vector.tensor_tensor(hh[:], xt[:], lnb[:], OP.mult)
                    nc.vector.tensor_scalar_mul(hh[:], hh[:], inv[:])
                    hf = None
                    if logf_fn is not None:
                        hf = io.tile([P, NDK, P], F32, tag=f"hf{tag}")
                    for kc in range(NDK):
                        pst = tp.tile([P, P], F32, tag="pstn")
                        _transpose(nc, pst[:], hh[:, kc * P:(kc + 1) * P], ident)
                        nc.any.tensor_copy(
                            out=dst[:, kc, blk * P:(blk + 1) * P], in_=pst[:])
                        if hf is not None:
                            nc.any.tensor_copy(out=hf[:, kc, :], in_=pst[:])
                    if logf_fn is not None:
                        logf_fn(blk, hf)

            with tc.tile_pool(name="lnp", bufs=1) as lnp, \
                 tc.tile_pool(name="hpool", bufs=1) as hpool, \
                 tc.tile_pool(name="wB", bufs=2) as wpoolB:
                ln1b = lnp.tile([P, D], F32)
                nc.sync.dma_start(ln1b[:1, :], ln1)
                nc.gpsimd.partition_broadcast(ln1b[:], ln1b[:1, :])

                hT = hpool.tile([P, NDK, S], BF16)
                hTq = hpool.tile([P, NDK, SQ], BF16)

                wft = lnp.tile([P, NDK, H], F32)
                nc.sync.dma_start(wft[:], wf3)

                with tc.tile_pool(name="ioA", bufs=2) as io, \
                     tc.tile_pool(name="tpA", bufs=4, space="PSUM") as tp, \
                     tc.tile_pool(name="psL", bufs=2, space="PSUM") as psl:
                    def logf_fn(blk, hf):
                        # forget gate from the fp32 staging of h^T: the
                        # decay path must stay fp32 (errors integrate over
                        # ~1000 cumsum steps before exp()).
                        pl = psl.tile([P, H], F32, tag="psl")
                        for dk in range(NDK):
                            _mm(nc, pl[:], hf[:, dk, :], wft[:, dk, :],
                                start=(dk == 0), stop=(dk == NDK - 1),
                                fast=False)
                        nc.vector.tensor_tensor(spn[:, blk, :], pl[:],
                                                bfb[:], OP.add)

                    norm_transpose(io, tp, xb, hT, NBLK, ln1b, "a",
                                   logf_fn=logf_fn)
                    norm_transpose(io, tp, xq, hTq, NQB, ln1b, "b")
                    # batched softplus(-z) = ln(1 + exp(-z)) over all blocks:
                    # exactly one Exp->Ln table round-trip for the whole phase
                    nc.scalar.activation(spn[:], spn[:], AT.Exp, scale=-1.0)
                    nc.scalar.activation(spn[:], spn[:], AT.Ln, bias=1.0)

                with tc.tile_pool(name="ioB", bufs=3) as io, \
                     tc.tile_pool(name="psB", bufs=4, space="PSUM") as ps, \
                     tc.tile_pool(name="tpB", bufs=4, space="PSUM") as tp:
                    wpool = wpoolB

                    # cumulative decay: csp[h, i] = sum_{j<=i} spn[j, h]
                    for it in range(2):
                        pc = ps.tile([P, 512], F32, tag="psb")
                        for blk in range(NBLK):
                            ut = io.tile([P, 512], F32, tag="ut")
                            nc.sync.dma_start(
                                ut[:], uc[blk * P:(blk + 1) * P,
                                          it * 512:(it + 1) * 512])
                            _mm(nc, pc[:H, :], spn[:, blk, :], ut[:],
                                start=(blk == 0), stop=(blk == NBLK - 1),
                                fast=False)
                        nc.any.tensor_copy(out=csp[:, it * 512:(it + 1) * 512],
                                           in_=pc[:H, :])
                    nc.sync.dma_start(csp_d[:], csp[:])

                    # q/k/v projections, 512-wide dout tiles (4 heads each)
                    def qk_proj(w3ap, hsrc, nblocks, nwb, dst_dram):
                        for dt in range(4):
                            wt = wpool.tile([P, NDK, 512], BF16, tag="wt")
                            nc.sync.dma_start(
                                wt[:], w3ap[:, :, dt * 512:(dt + 1) * 512])
                            for blk in range(nblocks):
                                pp = ps.tile([P, 512], F32, tag="psb")
                                for dk in range(NDK):
                                    _mm(nc, pp[:],
                                        hsrc[:, dk, blk * P:(blk + 1) * P],
                                        wt[:, dk, :], start=(dk == 0),
                                        stop=(dk == NDK - 1))
                                # all 4 heads of this tile processed with
                                # full-width [128,512] ops
                                qs = io.tile([P, 4, HD], F32, tag="qs")
                                nc.any.tensor_copy(out=qs[:], in_=pp[:])
                                sqt = io.tile([P, 4, HD], F32, tag="sqt")
                                nc.vector.tensor_tensor(sqt[:], qs[:], qs[:],
                                                        OP.mult)
                                ssq = io.tile([P, 4], F32, tag="ssqh")
                                nc.vector.reduce_sum(ssq[:], sqt[:],
                                                     axis=mybir.AxisListType.X)
                                inv = io.tile([P, 4], F32, tag="invh")
                                nc.vector.tensor_scalar(
                                    inv[:], ssq[:], 1.0 / HD, EPS,
                                    OP.mult, OP.add)
                                nc.scalar.activation(inv[:], inv[:], AT.Sqrt)
                                nc.vector.reciprocal(inv[:], inv[:])
                                nc.vector.tensor_tensor(
                                    qs[:], qs[:],
                                    inv[:, :, None].to_broadcast([P, 4, HD]),
                                    OP.mult)
                                nc.vector.tensor_tensor(qs[:], qs[:], nwb[:],
                                                        OP.mult)
                                kst = io.tile([P, 4, P], BF16, tag="kst")
                                for hh in range(4):
                                    pst = tp.tile([P, P], F32, tag="pstb")
                                    _transpose(nc, pst[:], qs[:, hh, :], ident)
                                    nc.any.tensor_copy(out=kst[:, hh, :],
                                                       in_=pst[:])
                                nc.sync.dma_start(
                                    dst_dram[dt * 4:(dt + 1) * 4, :,
                                             blk * P:(blk + 1) * P]
                                    .rearrange("h p f -> p h f"),
                                    kst[:])

                    qk_proj(wq3, hTq, NQB, qnb, qT_d)
                    qk_proj(wk3, hT, NBLK, knb, kT_d)

                    for dt in range(4):
                        wt = wpool.tile([P, NDK, 512], BF16, tag="wt")
                        nc.sync.dma_start(wt[:],
                                          wv3[:, :, dt * 512:(dt + 1) * 512])
                        for blk in range(NBLK):
                            pp = ps.tile([P, 512], F32, tag="psb")
                            for dk in range(NDK):
                                _mm(nc, pp[:], hT[:, dk, blk * P:(blk + 1) * P],
                                    wt[:, dk, :], start=(dk == 0),
                                    stop=(dk == NDK - 1))
                            vs = io.tile([P, 512], BF16, tag="vs")
                            nc.any.tensor_copy(out=vs[:], in_=pp[:])
                            nc.sync.dma_start(
                                v_d[blk, :, dt * 512:(dt + 1) * 512], vs[:])

            # ---------------- Phase C+D: attention + out-proj ----------
            with tc.tile_pool(name="opool", bufs=1) as opool, \
                 tc.tile_pool(name="wD", bufs=2) as wpoolD:
                oT = opool.tile([P, H, SQ], BF16)

                with tc.tile_pool(name="ioC", bufs=2) as io, \
                     tc.tile_pool(name="sC", bufs=3) as sc, \
                     tc.tile_pool(name="ptC", bufs=2) as ptp, \
                     tc.tile_pool(name="psC", bufs=3, space="PSUM") as ps, \
                     tc.tile_pool(name="tpC", bufs=2, space="PSUM") as tp, \
                     tc.tile_pool(name="opC", bufs=2, space="PSUM") as op:
                    for head in range(H):
                        crep = io.tile([P, S], F32, tag="crep")
                        nc.sync.dma_start(
                            crep[:], csp_d[head:head + 1, :].broadcast_to((P, S)))
                        kth = io.tile([P, S], BF16, tag="kth")
                        nc.sync.dma_start(kth[:], kT_d[head])
                        qth = io.tile([P, SQ], BF16, tag="qth")
                        nc.sync.dma_start(qth[:], qT_d[head])
                        vh = io.tile([P, NBLK, HD], BF16, tag="vh")
                        nc.sync.dma_start(
                            vh[:], v_d[:, :, head * HD:(head + 1) * HD]
                            .rearrange("b p f -> p b f"))
                        ptb = ptp.tile([P, NBLK, SQ], BF16, tag="ptb")

                        for t in range(NQB):
                            nk = 256 * (t + 1)
                            nparts = (nk + 511) // 512
                            sps = []
                            for prt in range(nparts):
                                w = min(512, nk - prt * 512)
                                pt = ps.tile([P, 512], F32, tag="spsum")
                                _mm(nc, pt[:, :w], qth[:, t * P:(t + 1) * P],
                                    kth[:, prt * 512:prt * 512 + w])
                                sps.append((pt, w))
                            sp = sc.tile([P, S], F32, tag="sprime")
                            for prt, (pt, w) in enumerate(sps):
                                nc.vector.tensor_tensor(
                                    sp[:, prt * 512:prt * 512 + w], pt[:, :w],
                                    crep[:, prt * 512:prt * 512 + w], OP.add)
                            nc.vector.tensor_tensor(
                                sp[:, nk - 256:nk], sp[:, nk - 256:nk],
                                mtile[:], OP.add)
                            mx = io.tile([P, 1], F32, tag="mx")
                            nc.vector.reduce_max(mx[:], sp[:, :nk],
                                                 axis=mybir.AxisListType.X)
                            nc.vector.tensor_scalar_mul(mx[:], mx[:], -1.0)
                            pe = sc.tile([P, S], F32, tag="pexp")
                            rs = io.tile([P, 1], F32, tag="rs")
                            nc.scalar.activation(pe[:, :nk], sp[:, :nk],
                                                 AT.Exp, bias=mx[:],
                                                 accum_out=rs[:])
                            nc.vector.reciprocal(rs[:], rs[:])
                            nc.vector.tensor_scalar_mul(pe[:, :nk],
                                                        pe[:, :nk], rs[:])
                            for kc in range(2 * t + 2):
                                pst = tp.tile([P, P], F32, tag="pstc")
                                _transpose(nc, pst[:],
                                           pe[:, kc * P:(kc + 1) * P], ident)
                                nc.any.tensor_copy(
                                    out=ptb[:, kc, t * P:(t + 1) * P],
                                    in_=pst[:])

                        # Suffix accumulation: key-chunk kc only feeds query
                        # slots t >= kc//2. kc=0 (start=True) clears the whole
                        # bank and covers every column, so unwritten regions
                        # of later chunks are never read.
                        po = op.tile([P, SQ], F32, tag="opsum")
                        for kc in range(NBLK):
                            qlo = (kc // 2) * P
                            _mm(nc, po[:, qlo:], vh[:, kc, :], ptb[:, kc, qlo:],
                                start=(kc == 0), stop=(kc == NBLK - 1))
                        nc.any.tensor_copy(out=oT[:, head, :], in_=po[:])

                with tc.tile_pool(name="ioD", bufs=3) as io, \
                     tc.tile_pool(name="psD", bufs=4, space="PSUM") as ps:
                    wpool = wpoolD
                    for dt in range(4):
                        wt = wpool.tile([P, NDK, 512], BF16, tag="wod")
                        nc.sync.dma_start(wt[:],
                                          wo3[:, :, dt * 512:(dt + 1) * 512])
                        for slot in range(NQB):
                            pp = ps.tile([P, 512], F32, tag="psd")
                            for hh in range(H):
                                _mm(nc, pp[:],
                                    oT[:, hh, slot * P:(slot + 1) * P],
                                    wt[:, hh, :], start=(hh == 0),
                                    stop=(hh == H - 1))
                            xqt = io.tile([P, 512], F32, tag="xqt")
                            nc.sync.dma_start(
                                xqt[:], xq[slot * P:(slot + 1) * P,
                                           dt * 512:(dt + 1) * 512])
                            x2t = io.tile([P, 512], F32, tag="x2t")
                            nc.vector.tensor_tensor(x2t[:], pp[:], xqt[:],
                                                    OP.add)
                            nc.sync.dma_start(
                                x2_d[slot * P:(slot + 1) * P,
                                     dt * 512:(dt + 1) * 512], x2t[:])

            # ---------------- Phase E: SwiGLU MLP ----------------------
            with tc.tile_pool(name="mpool", bufs=1) as mpool, \
                 tc.tile_pool(name="ln2p", bufs=1) as ln2p, \
                 tc.tile_pool(name="wE", bufs=3) as wpoolE:
                ln2b = ln2p.tile([P, D], F32)
                nc.sync.dma_start(ln2b[:1, :], ln2)
                nc.gpsimd.partition_broadcast(ln2b[:], ln2b[:1, :])

                h2T = mpool.tile([P, NDK, SQ], BF16)
                mT = mpool.tile([P, NFC, SQ], BF16)

                with tc.tile_pool(name="ioE", bufs=2) as io, \
                     tc.tile_pool(name="tpE", bufs=4, space="PSUM") as tp:
                    norm_transpose(io, tp, x2_d, h2T, NQB, ln2b, "e")

                with tc.tile_pool(name="ioF", bufs=3) as io, \
                     tc.tile_pool(name="psGU", bufs=2, space="PSUM") as psgu, \
                     tc.tile_pool(name="psY", bufs=1, space="PSUM") as psy:
                    wpool = wpoolE
                    for fc in range(NFC):
                        w1t = wpool.tile([P, NDK, P], BF16, tag="w1t")
                        nc.sync.dma_start(w1t[:], w13[:, :, fc * P:(fc + 1) * P])
                        w2t = wpool.tile([P, NDK, P], BF16, tag="w2t")
                        nc.sync.dma_start(w2t[:], w23[:, :, fc * P:(fc + 1) * P])
                        pg = psgu.tile([P, SQ], F32, tag="pge")
                        for dk in range(NDK):
                            _mm(nc, pg[:], w1t[:, dk, :], h2T[:, dk, :],
                                start=(dk == 0), stop=(dk == NDK - 1))
                        pu = psgu.tile([P, SQ], F32, tag="pue")
                        for dk in range(NDK):
                            _mm(nc, pu[:], w2t[:, dk, :], h2T[:, dk, :],
                                start=(dk == 0), stop=(dk == NDK - 1))
                        gs = io.tile([P, SQ], F32, tag="gse")
                        nc.scalar.activation(gs[:], pg[:], AT.Silu)
                        nc.vector.tensor_tensor(mT[:, fc, :], gs[:], pu[:],
                                                OP.mult)

                    for dt in range(4):
                        yps = [psy.tile([P, 512], F32, tag=f"ype{slot}",
                                        name=f"yps{dt}_{slot}")
                               for slot in range(NQB)]
                        for fc in range(NFC):
                            w3t = wpool.tile([P, 512], BF16, tag="w3t")
                            nc.sync.dma_start(
                                w3t[:], w3[fc * P:(fc + 1) * P,
                                           dt * 512:(dt + 1) * 512])
                            for slot in range(NQB):
                                _mm(nc, yps[slot][:],
                                    mT[:, fc, slot * P:(slot + 1) * P], w3t[:],
                                    start=(fc == 0), stop=(fc == NFC - 1))
                        for slot in range(NQB):
                            x2t = io.tile([P, 512], F32, tag="x2r")
                            nc.sync.dma_start(
                                x2t[:], x2_d[slot * P:(slot + 1) * P,
                                             dt * 512:(dt + 1) * 512])
                            ot = io.tile([P, 512], F32, tag="ote")
                            nc.vector.tensor_tensor(ot[:], yps[slot][:],
                                                    x2t[:], OP.add)
                            nc.sync.dma_start(
                                out[slot * P:(slot + 1) * P,
                                    dt * 512:(dt + 1) * 512], ot[:])

    nc.compile()
    return nc


def _get_nc():
    global _NC
    if _NC is None:
        _NC = build_nc()
    return _NC


def _host_prep(inputs):
    import ml_dtypes
    bf = ml_dtypes.bfloat16
    x = np.ascontiguousarray(np.asarray(inputs["x"], dtype=np.float32))
    w1p = np.zeros((D, FFP), bf)
    w1p[:, :FF] = np.asarray(inputs["w1"]).astype(bf)
    w2p = np.zeros((D, FFP), bf)
    w2p[:, :FF] = np.asarray(inputs["w2"]).astype(bf)
    w3p = np.zeros((FFP, D), bf)
    w3p[:FF, :] = np.asarray(inputs["w3"]).astype(bf)
    uconst = np.triu(np.ones((S, S), np.float32))
    masks = []
    for p in range(2):
        r = np.arange(P)[:, None]
        f = np.arange(256)[None, :]
        masks.append(np.where(f <= 2 * r + p, 0.0, NEG).astype(np.float32))
    shared = {
        "wq": np.asarray(inputs["wq"]).astype(bf),
        "wk": np.asarray(inputs["wk"]).astype(bf),
        "wv": np.asarray(inputs["wv"]).astype(bf),
        "wo": np.asarray(inputs["wo"]).astype(bf),
        "wf": np.ascontiguousarray(inputs["wf"], dtype=np.float32),
        "bfv": np.asarray(inputs["bf"], dtype=np.float32).reshape(1, H),
        "qn": (np.asarray(inputs["qn_w"], dtype=np.float32)
               / np.sqrt(HD).astype(np.float32)).reshape(1, HD),
        "kn": np.asarray(inputs["kn_w"], dtype=np.float32).reshape(1, HD),
        "ln1": np.asarray(inputs["ln1_w"], dtype=np.float32).reshape(1, D),
        "ln2": np.asarray(inputs["ln2_w"], dtype=np.float32).reshape(1, D),
        "w1": w1p, "w2": w2p, "w3": w3p, "uc": uconst,
    }
    in_maps = []
    for c in range(8):
        b, p = c // 2, c % 2
        m = dict(shared)
        m["xb"] = np.ascontiguousarray(x[b])
        m["xq"] = np.ascontiguousarray(x[b][p::2])
        m["msk"] = masks[p]
        in_maps.append(m)
    return in_maps


def run(inputs, trace=False, tmpdir=None):
    nc = _get_nc()
    in_maps = _host_prep(inputs)
    res = bass_utils.run_bass_kernel_spmd(
        nc, in_maps, core_ids=list(range(8)), trace=trace, tmpdir=tmpdir)
    out = np.empty((B, S, D), np.float32)
    for c in range(8):
        b, p = c // 2, c % 2
        out[b, p::2, :] = res.results[c]["out"]
    return out, res


def kernel(**inputs):
    out, _ = run(inputs, trace=False)
    return out
